# revision 11
# baseline (speedup 1.0000x reference)
"""Trainium2 Bass kernel for nn_CurvatureOnlyRegularizer (retrieval_knn).

Full inputs -> full output. Shards the 8192 points row-wise across 8 cores.

Per-core pipeline (1024 rows = 8 row-tiles of 128), software-pipelined per
tile:
  A. S = e1 . e2^T via bf16 PE matmul (4 K-chunks, 2 column-chunks of 512 per
     PSUM group).  ACT evacuates as t1 = Relu(psum*25 + bias_i) with bias_i
     folding the per-row term and the 1.5*2^23 magic constant, so t1 holds an
     integer m_i-part in fp32's integer binade.
  B. One scalar_tensor_tensor adds the per-chunk table (-25*n_j quantized +
     idx_j/512), producing packed = m + idx/512 with m = round(25*(C0-d^2)),
     |m| < 2^15 for every real neighbor so the packed value is exact.  Per-512-chunk max8 carries indices through selection for free;
     top-16-of-candidates + max_index recovers (chunk, idx) -> global idx.
  C. Curvature signature from the quantized d'^2; inv-distances stashed for
     the cosine stage.
  D. Neighbor embeddings gathered via dma_gather(transpose=True) into
     [D-partition, point*16] layout; PE gram (4 K-chunks x 16 col-groups)
     gives all pairwise dots incl. the self row/col.
  E. The gram PSUM is ACT-evacuated to fp16 and ONE 4-dim-AP DMA per (tile,
     half) scatters the 16x16 block-diagonals directly into per-point rows
     (ptR[point, l*16+m]) - no separate fold pass.
  F. cos = (G - G_l15 - G_r15 + G_1515) * invd_l * invd_m; upper-triangle via
     ap_gather; signatures sorted descending by max8/match_replace rounds and
     MSE'd against host-reversed references on ACT.  Phase F of tile t runs
     under phase A of tile t+2.
Host sums the 8 per-core partial sums.
"""

import os
from contextlib import ExitStack

import ml_dtypes
import numpy as np

import concourse.bass as bass
import concourse.bass_isa as bass_isa
import concourse.mybir as mybir
import concourse.tile as tile
from concourse import bacc
from concourse.bass import ds, ts
from concourse.bass_utils import run_bass_kernel_spmd

N, D, K = 8192, 512, 15
NCORES = 8
SHARD = N // NCORES            # 1024
RT = SHARD // 128              # 8 row-tiles per core
NCH = N // 512                 # 16 column chunks
NG = NCH // 2                  # 8 psum groups of 2 chunks per row-tile
MAGIC = 12582912.0             # 1.5 * 2^23
C0 = 2052.0
QSCALE = 25.0
PAD_CURV = -1.0
PAD_ANG = -4.0
NEG_BIG = -3.0e38
F32 = mybir.dt.float32
F16 = mybir.dt.float16
BF16 = mybir.dt.bfloat16
I16 = mybir.dt.int16
U32 = mybir.dt.uint32
AX = mybir.AxisListType
OP = mybir.AluOpType
AF = mybir.ActivationFunctionType

# which of the 8 psum groups run their pack-STT on gpsimd (rest on vector);
# gpsimd lacks the fused scalar_tensor_tensor opcode, so this must stay empty
STT_ON_GPSIMD = ()


def build_nc(debug_out: bool = False):
    nc = bacc.Bacc("TRN2", target_bir_lowering=False, debug=False)

    rhsT_d = nc.dram_tensor("rhsT", [D, N], BF16, kind="ExternalInput")
    lhsT_d = nc.dram_tensor("lhsT", [D, SHARD], BF16, kind="ExternalInput")
    egat_d = nc.dram_tensor("egather", [N, D], BF16, kind="ExternalInput")
    nji_d = nc.dram_tensor("njiota", [128, N], F32, kind="ExternalInput")
    bias_d = nc.dram_tensor("bias", [128, RT], F32, kind="ExternalInput")
    self_d = nc.dram_tensor("selfidx", [128, RT], F32, kind="ExternalInput")
    refc_d = nc.dram_tensor("refc", [128, RT * 16], F32, kind="ExternalInput")
    refa_d = nc.dram_tensor("refa", [128, RT * 112], F32, kind="ExternalInput")
    triu_d = nc.dram_tensor("triu", [128, 7], I16, kind="ExternalInput")
    fold_d = nc.dram_tensor("foldbuf", [RT, 128, 256], F16, kind="Internal")
    part_d = nc.dram_tensor("partial", [1, 2], F32, kind="ExternalOutput")
    if debug_out:
        dbg_idx_d = nc.dram_tensor("dbg_idx", [128, 16], F32, kind="ExternalOutput")
        dbg_d2_d = nc.dram_tensor("dbg_d2", [128, 16], F32, kind="ExternalOutput")
        dbg_srtc_d = nc.dram_tensor("dbg_srtc", [128, 16], F32, kind="ExternalOutput")
        dbg_ang_d = nc.dram_tensor("dbg_ang", [128, 112], F32, kind="ExternalOutput")
        dbg_cand_d = nc.dram_tensor("dbg_cand", [128, 128], F32, kind="ExternalOutput")
        dbg_ptr_d = nc.dram_tensor("dbg_ptr", [128, 256], F32, kind="ExternalOutput")

    # preamble (before Tile body): the gather-count register
    r2048 = nc.gpsimd.to_reg(2048)
    rfill1 = nc.gpsimd.to_reg(1.0)

    with tile.TileContext(nc) as tc, ExitStack() as ctx:
        const = ctx.enter_context(tc.tile_pool(name="const", bufs=1))
        sel = ctx.enter_context(tc.tile_pool(name="sel", bufs=3))
        scr = ctx.enter_context(tc.tile_pool(name="scr", bufs=3))
        fb = ctx.enter_context(tc.tile_pool(name="fb", bufs=3))
        gskp = ctx.enter_context(tc.tile_pool(name="gskp", bufs=2))
        vbuf = ctx.enter_context(tc.tile_pool(name="vbuf", bufs=2))
        ptrp = ctx.enter_context(tc.tile_pool(name="ptrp", bufs=3))
        ivdp = ctx.enter_context(tc.tile_pool(name="ivdp", bufs=3))
        psA = ctx.enter_context(tc.tile_pool(name="psA", bufs=2, space="PSUM"))
        psG = ctx.enter_context(tc.tile_pool(name="psG", bufs=1, space="PSUM"))
        psT = ctx.enter_context(tc.tile_pool(name="psT", bufs=1, space="PSUM"))

        # ---- constants / resident data ----
        rhs_sb = [const.tile([128, N], BF16, tag=f"rhs{c}", name=f"rhs{c}") for c in range(4)]
        lhs_sb = [const.tile([128, SHARD], BF16, tag=f"lhs{c}", name=f"lhs{c}") for c in range(4)]
        nji_sb = const.tile([128, N], F32, tag="nji")
        bias_sb = const.tile([128, RT], F32, tag="bias")
        self_sb = const.tile([128, RT], F32, tag="self")
        refc_sb = const.tile([128, RT * 16], F32, tag="refc")
        refa_sb = const.tile([128, RT * 112], F32, tag="refa")
        triu_sb = const.tile([128, 7], I16, tag="triu")
        perm = const.tile([128, 128], F32, tag="perm")
        repmat = const.tile([16, 128], F32, tag="repmat")
        css = const.tile([128, 1], F32, tag="css")
        ass = const.tile([128, 1], F32, tag="ass")

        for c in range(4):
            nc.sync.dma_start(rhs_sb[c][:], rhsT_d.ap()[ts(c, 128), :])
            nc.sync.dma_start(lhs_sb[c][:], lhsT_d.ap()[ts(c, 128), :])
        nc.sync.dma_start(nji_sb[:], nji_d.ap()[:])
        nc.sync.dma_start(bias_sb[:], bias_d.ap()[:])
        nc.sync.dma_start(self_sb[:], self_d.ap()[:])
        nc.sync.dma_start(triu_sb[:], triu_d.ap()[:])
        nc.sync.dma_start(refc_sb[:], refc_d.ap()[:])
        nc.sync.dma_start(refa_sb[:], refa_d.ap()[:])
        # perm[k, 8u+v] = 1 iff k == 16v+u  (gram-position permutation)
        nc.gpsimd.memset(perm[:], 0.0)
        nc.gpsimd.affine_select(
            out=bass.AP(tensor=perm[:].tensor, offset=0,
                        ap=[perm[:].ap[0], [8, 16], [1, 8]]),
            in_=bass.AP(tensor=perm[:].tensor, offset=0,
                        ap=[perm[:].ap[0], [8, 16], [1, 8]]),
            compare_op=OP.not_equal,
            fill=rfill1, base=0,
            pattern=[[-1, 16], [-16, 8]],
            channel_multiplier=1,
        )
        # repmat[k, 16a+s] = 1 iff k == s  (partition-block replicator)
        nc.gpsimd.memset(repmat[:], 0.0)
        nc.gpsimd.affine_select(
            out=bass.AP(tensor=repmat[:].tensor, offset=0,
                        ap=[repmat[:].ap[0], [16, 8], [1, 16]]),
            in_=bass.AP(tensor=repmat[:].tensor, offset=0,
                        ap=[repmat[:].ap[0], [16, 8], [1, 16]]),
            compare_op=OP.not_equal,
            fill=rfill1, base=0,
            pattern=[[0, 8], [-1, 16]],
            channel_multiplier=1,
        )
        nc.vector.memset(css[:], 0.0)
        nc.vector.memset(ass[:], 0.0)

        # =========== phase A: matmul + pack + chunk-max8 ===========
        def phase_a(t):
            cand = sel.tile([128, 128], F32, tag="cand")
            for g in range(NG):
                ps = psA.tile([128, 1024], F32, tag="psA", name="psA")
                for c in range(4):
                    for q in range(2):
                        nc.tensor.matmul(
                            ps[:, ts(q, 512)],
                            lhs_sb[c][:, ts(t, 128)],
                            rhs_sb[c][:, ts(2 * g + q, 512)],
                            start=(c == 0),
                            stop=(c == 3),
                        )
                t1 = scr.tile([128, 1024], F32, tag="t1")
                nc.scalar.activation(
                    t1[:], ps[:], AF.Relu,
                    bias=bias_sb[:, t : t + 1], scale=QSCALE,
                )
                t2 = scr.tile([128, 1024], F32, tag="t2")
                eng = nc.gpsimd if g in STT_ON_GPSIMD else nc.vector
                eng.scalar_tensor_tensor(
                    t2[:], t1[:], -MAGIC, nji_sb[:, ts(g, 1024)],
                    op0=OP.add, op1=OP.add,
                )
                for q in range(2):
                    nc.vector.max(cand[:, ts(2 * g + q, 8)], t2[:, ts(q, 512)])
            return cand

        # =========== phase B: select top-16 + unpack; C: curvature ==
        def phase_bc(t, cand):
            v16 = sel.tile([128, 16], F32, tag="v16")
            nc.vector.max(v16[:, 0:8], cand[:])
            candz = sel.tile([128, 128], F32, tag="candz")
            nc.vector.match_replace(candz[:], v16[:, 0:8], cand[:], NEG_BIG)
            nc.vector.max(v16[:, 8:16], candz[:])
            pos = sel.tile([128, 16], U32, tag="pos")
            nc.vector.max_index(pos[:, 0:8], v16[:, 0:8], cand[:])
            nc.vector.max_index(pos[:, 8:16], v16[:, 8:16], candz[:])
            chunk_u = sel.tile([128, 16], U32, tag="chunku")
            nc.vector.tensor_scalar(
                chunk_u[:], pos[:], 3, None, op0=OP.logical_shift_right
            )
            chunk_f = sel.tile([128, 16], F32, tag="chunkf")
            nc.vector.tensor_copy(chunk_f[:], chunk_u[:])
            # unpack m (integer part) via magic round
            s1 = sel.tile([128, 16], F32, tag="s1")
            nc.vector.tensor_scalar(
                s1[:], v16[:], -0.4990234375, None, op0=OP.add
            )
            wv = sel.tile([128, 16], F32, tag="wv")
            nc.scalar.activation(wv[:], s1[:], AF.Copy, bias=MAGIC, scale=1.0)
            m16 = sel.tile([128, 16], F32, tag="m16")
            nc.vector.tensor_scalar(m16[:], wv[:], -MAGIC, None, op0=OP.add)
            # frac = v16 - m16 = idx/512 ; gidx = chunk*512 + frac*512
            frac = sel.tile([128, 16], F32, tag="frac")
            nc.vector.scalar_tensor_tensor(
                frac[:], m16[:], -1.0, v16[:], op0=OP.mult, op1=OP.add
            )
            gidx = sel.tile([128, 16], F32, tag="gidx")
            nc.vector.tensor_tensor(gidx[:], chunk_f[:], frac[:], op=OP.add)
            nc.vector.tensor_scalar(gidx[:], gidx[:], 512.0, None, op0=OP.mult)
            # dp2 = C0 - m/QSCALE (slot 0 = self, dropped)
            dp2 = sel.tile([128, 16], F32, tag="dp2")
            nc.vector.tensor_scalar(
                dp2[:], m16[:], -1.0 / QSCALE, C0, op0=OP.mult, op1=OP.add
            )
            # ---- phase C: curvature ----
            d2re = sel.tile([128, 16], F32, tag="d2re")
            nc.vector.tensor_scalar_max(d2re[:, 0:15], dp2[:, 1:16], 1e-12)
            nc.vector.memset(d2re[:, 15:16], 1.0)
            dt_ = sel.tile([128, 16], F32, tag="dt")
            nc.scalar.sqrt(dt_[:], d2re[:])
            ivd = ivdp.tile([128, 16], F32, tag="ivd")
            nc.vector.reciprocal(ivd[:], dt_[:])
            dsum = sel.tile([128, 1], F32, tag="dsum")
            nc.vector.reduce_sum(dsum[:], dt_[:, 0:15], axis=AX.X)
            dmean = sel.tile([128, 1], F32, tag="dmean")
            nc.vector.tensor_scalar(
                dmean[:], dsum[:], 1.0 / 15.0, 1e-8, op0=OP.mult, op1=OP.add
            )
            ivm = sel.tile([128, 1], F32, tag="ivm")
            nc.vector.reciprocal(ivm[:], dmean[:])
            sig = sel.tile([128, 16], F32, tag="sig")
            nc.vector.tensor_scalar(
                sig[:, 0:15], dt_[:, 0:15], ivm[:], None, op0=OP.mult
            )
            nc.vector.memset(sig[:, 15:16], PAD_CURV)
            srtc = sel.tile([128, 16], F32, tag="srtc")
            nc.vector.max(srtc[:, 0:8], sig[:])
            sigz = sel.tile([128, 16], F32, tag="sigz")
            nc.vector.match_replace(sigz[:], srtc[:, 0:8], sig[:], -2.0)
            nc.vector.max(srtc[:, 8:16], sigz[:])
            dcv = sel.tile([128, 16], F32, tag="dcv")
            nc.vector.tensor_tensor(
                dcv[:], srtc[:], refc_sb[:, ts(t, 16)], op=OP.subtract
            )
            csq = sel.tile([128, 16], F32, tag="csq")
            css_t = sel.tile([128, 1], F32, tag="csst")
            nc.scalar.activation(csq[:], dcv[:], AF.Square, accum_out=css_t[:])
            nc.vector.tensor_tensor(css[:], css[:], css_t[:], op=OP.add)
            # ---- neighbor index tile for the gather ----
            kif = sel.tile([128, 16], F32, tag="kif")
            nc.vector.tensor_copy(kif[:, 0:15], gidx[:, 1:16])
            nc.vector.tensor_copy(kif[:, 15:16], self_sb[:, t : t + 1])
            # idx16[16a+s, Q] = kif[phi(Q), s]: transpose (perm rhs),
            # then replicate the 16-row block via a constant matmul.
            pst1 = psT.tile([16, 128], F32, tag="pst1", name="pst1")
            nc.tensor.transpose(pst1[:], kif[:], perm[:])
            t1s = sel.tile([16, 128], F32, tag="t1s")
            nc.vector.tensor_copy(t1s[:], pst1[:])
            pst = psT.tile([128, 128], F32, tag="pst", name="pst")
            nc.tensor.matmul(pst[:], repmat[:], t1s[:], start=True, stop=True)
            idx16 = sel.tile([128, 128], I16, tag="idx16")
            nc.vector.tensor_copy(idx16[:], pst[:])
            return ivd, kif, dp2, idx16, srtc, cand

        def gather(idx16):
            vt = vbuf.tile([128, 4, 2048], BF16, tag="vt")
            nc.gpsimd.dma_gather(
                out_ap=vt[:],
                in_ap=egat_d.ap()[:],
                idxs_ap=idx16[:],
                num_idxs=2048,
                num_idxs_reg=r2048,
                elem_size=512,
                transpose=True,
                single_packet=False,
            )
            return vt

        # =========== phase D/E: gram + fused fold scatter ===========
        def gram_fold(t, vt):
            ptr_t = ptrp.tile([128, 256], F16, tag="ptr")
            gsk = gskp.tile([128, 2048], F16, tag="gsk", name="gsk")
            for w in range(2):
                pg = psG.tile([128, 1024], F32, tag="pg", name="pg")
                for g2h in range(8):
                    g2 = 8 * w + g2h
                    for c in range(4):
                        nc.tensor.matmul(
                            pg[:, ts(g2h, 128)],
                            vt[:, c, ts(g2, 128)],
                            vt[:, c, ts(g2, 128)],
                            start=(c == 0),
                            stop=(c == 3),
                        )
                nc.scalar.activation(gsk[:, ts(w, 1024)], pg[:], AF.Copy)
            # fold via a DRAM bounce (DRAM APs have no partition-step rules):
            # per p: gsk[16p+l, 128*k + 16p + m] -> D[t, 16p+k, 16l+m]
            # (diag extraction + l<->k block transpose in one DMA), then one
            # contiguous DMA D[t] -> ptr_t.
            for p in range(8):
                src = bass.AP(
                    tensor=gsk[:].tensor,
                    offset=gsk[:].offset + p * (16 * 2048 + 16),
                    ap=[[2048, 16], [128, 16], [1, 16]],
                )
                dst = bass.AP(
                    tensor=fold_d.ap()[:].tensor,
                    offset=t * 128 * 256 + p * 16 * 256,
                    ap=[[16, 16], [256, 16], [1, 16]],
                )
                eng = nc.sync if p % 2 == 0 else nc.scalar
                eng.dma_start(dst, src)
            nc.sync.dma_start(ptr_t[:], fold_d.ap()[t])
            return ptr_t

        # =========== phase F: cos + sort + MSE ======================
        def phase_f(t, ptr_t, ivd, dbg=False):
            p0 = ptr_t[:].ap[0]
            base = ptr_t[:].offset
            pr = bass.AP(tensor=ptr_t[:].tensor, offset=base,
                         ap=[p0, [16, 16], [1, 16]])
            in_l15 = bass.AP(tensor=ptr_t[:].tensor, offset=base + 15,
                             ap=[p0, [16, 16], [0, 16]])
            in_r15 = bass.AP(tensor=ptr_t[:].tensor, offset=base + 240,
                             ap=[p0, [0, 16], [1, 16]])
            r1515 = bass.AP(tensor=ptr_t[:].tensor, offset=base + 255,
                            ap=[p0, [1, 1]])
            ta = fb.tile([128, 256], F32, tag="ta")
            nc.vector.tensor_tensor(ta[:], pr, in_l15, op=OP.subtract)
            tb = fb.tile([128, 256], F32, tag="tb")
            nc.vector.tensor_tensor(tb[:], ta[:], in_r15, op=OP.subtract)
            ivt = ivd[:]
            iv_l = bass.AP(
                tensor=ivt.tensor, offset=ivt.offset,
                ap=[ivt.ap[0], [1, 16], [0, 16]],
            )
            iv_m = bass.AP(
                tensor=ivt.tensor, offset=ivt.offset,
                ap=[ivt.ap[0], [0, 16], [1, 16]],
            )
            tcc = fb.tile([128, 256], F32, tag="tc")
            nc.vector.scalar_tensor_tensor(
                tcc[:], tb[:], r1515, iv_l, op0=OP.add, op1=OP.mult
            )
            cosv = fb.tile([128, 256], F32, tag="cosv")
            nc.vector.tensor_tensor(cosv[:], tcc[:], iv_m, op=OP.mult)
            angv = fb.tile([128, 112], F32, tag="angv")
            nc.gpsimd.ap_gather(
                out_ap=angv[:].rearrange("p (a b) -> p a b", b=1),
                in_ap=cosv[:].rearrange("p (a b) -> p a b", b=1),
                idxs_ap=triu_sb[:],
                channels=128,
                num_elems=256,
                d=1,
                num_idxs=112,
            )
            nc.vector.memset(angv[:, 105:112], PAD_ANG)
            srta = fb.tile([128, 112], F32, tag="srta")
            work = angv
            for r in range(14):
                nc.vector.max(srta[:, ts(r, 8)], work[:])
                if r < 13:
                    nwork = fb.tile([128, 112], F32, tag=f"work{r % 2}")
                    nc.vector.match_replace(
                        nwork[:], srta[:, ts(r, 8)], work[:], NEG_BIG
                    )
                    work = nwork
            dav = fb.tile([128, 112], F32, tag="dav")
            nc.vector.tensor_tensor(
                dav[:], srta[:], refa_sb[:, ts(t, 112)], op=OP.subtract
            )
            asq = fb.tile([128, 112], F32, tag="asq")
            ass_t = fb.tile([128, 1], F32, tag="asst")
            nc.scalar.activation(asq[:], dav[:], AF.Square, accum_out=ass_t[:])
            nc.vector.tensor_tensor(ass[:], ass[:], ass_t[:], op=OP.add)
            if dbg:
                nc.sync.dma_start(dbg_ang_d.ap()[:], srta[:])
                dbgp = fb.tile([128, 256], F32, tag="dbgp")
                nc.vector.tensor_copy(dbgp[:], ptr_t[:])
                nc.sync.dma_start(dbg_ptr_d.ap()[:], dbgp[:])

        # ================= main software-pipelined loop =============
        inflight = []   # list of (t, ptr_t, ivd) pending phase F
        gpend = []      # list of (t, vt) pending gram
        for t in range(RT):
            with nc.named_scope(f"A{t}"):
                cand = phase_a(t)
            with nc.named_scope(f"B{t}"):
                ivd, kif, dp2, idx16, srtc, cand_ = phase_bc(t, cand)
            with nc.named_scope(f"gth{t}"):
                vt = gather(idx16)
            if debug_out and t == 0:
                nc.sync.dma_start(dbg_idx_d.ap()[:], kif[:])
                nc.sync.dma_start(dbg_d2_d.ap()[:], dp2[:])
                nc.sync.dma_start(dbg_srtc_d.ap()[:], srtc[:])
                nc.sync.dma_start(dbg_cand_d.ap()[:], cand_[:])
            gpend.append((t, vt, ivd))
            if len(gpend) >= 2:
                tg, vtg, ivdg = gpend.pop(0)
                with nc.named_scope(f"G{tg}"):
                    ptr_t = gram_fold(tg, vtg)
                inflight.append((tg, ptr_t, ivdg))
            while len(inflight) >= 2:
                tf, ptr_f, ivd_f = inflight.pop(0)
                with nc.named_scope(f"F{tf}"):
                    phase_f(tf, ptr_f, ivd_f, dbg=(debug_out and tf == 0))
        while gpend:
            tg, vtg, ivdg = gpend.pop(0)
            with nc.named_scope(f"G{tg}"):
                ptr_t = gram_fold(tg, vtg)
            inflight.append((tg, ptr_t, ivdg))
        while inflight:
            tf, ptr_f, ivd_f = inflight.pop(0)
            with nc.named_scope(f"F{tf}"):
                phase_f(tf, ptr_f, ivd_f, dbg=(debug_out and tf == 0))

        # ---- final reduce + output ----
        cred = const.tile([128, 1], F32, tag="cred")
        ared = const.tile([128, 1], F32, tag="ared")
        nc.gpsimd.partition_all_reduce(
            cred[:], css[:], channels=128, reduce_op=bass_isa.ReduceOp.add
        )
        nc.gpsimd.partition_all_reduce(
            ared[:], ass[:], channels=128, reduce_op=bass_isa.ReduceOp.add
        )
        fin = const.tile([1, 2], F32, tag="fin")
        nc.vector.tensor_copy(fin[0:1, 0:1], cred[0:1, :])
        nc.vector.tensor_copy(fin[0:1, 1:2], ared[0:1, :])
        nc.sync.dma_start(part_d.ap()[:], fin[:])

    nc.compile()
    return nc


# =====================  host side  =====================

def _prep_inputs(embeddings, reference_curvature_sig, reference_angular_sig):
    emb32 = np.asarray(embeddings, dtype=np.float32)
    refc = np.asarray(reference_curvature_sig, dtype=np.float32)
    refa = np.asarray(reference_angular_sig, dtype=np.float32)

    e1_bf = (np.float32(np.sqrt(2.0)) * emb32).astype(ml_dtypes.bfloat16)
    e1 = e1_bf.astype(np.float32)
    e2_bf = (2.0 * e1).astype(ml_dtypes.bfloat16)       # exact x2
    n1 = np.sum(e1.astype(np.float64) * e1.astype(np.float64), axis=1).astype(
        np.float32
    )

    rhsT = np.ascontiguousarray(e2_bf.T)                # [512, N]
    lhsT_full = np.ascontiguousarray(e1_bf.T)           # [512, N]

    # packed per-chunk table: -round(25*n_j) + idx_within_chunk/512
    njq = np.round(QSCALE * n1.astype(np.float64))
    idx512 = np.tile(np.arange(512, dtype=np.float64) / 512.0, NCH)
    nji_row = (-njq + idx512).astype(np.float32)        # [N]
    nji = np.ascontiguousarray(
        np.broadcast_to(nji_row[None, :], (128, N))
    )

    tri = [l * 16 + m for l in range(15) for m in range(l + 1, 15)]
    tri += [255] * 7
    tri = np.array(tri, dtype=np.int16)                 # 112 entries
    triu = np.zeros((128, 7), dtype=np.int16)
    for p in range(128):
        for s in range(7):
            triu[p, s] = tri[s * 16 + (p & 15)]

    shared = dict(
        rhsT=rhsT, egather=e1_bf, njiota=nji, triu=triu,
    )
    per_core = []
    for c in range(NCORES):
        lo = c * SHARD
        sl = slice(lo, lo + SHARD)
        bias = (MAGIC + (C0 - n1[sl].astype(np.float64)) * QSCALE).astype(np.float32)
        bias_t = bias.reshape(RT, 128).T.copy()         # [128, RT]
        selfidx = (np.arange(lo, lo + SHARD, dtype=np.float32)
                   .reshape(RT, 128).T.copy())
        refc_c = np.full((SHARD, 16), PAD_CURV, dtype=np.float32)
        refc_c[:, 0:15] = refc[sl, ::-1]
        refa_c = np.full((SHARD, 112), PAD_ANG, dtype=np.float32)
        refa_c[:, 0:105] = refa[sl, ::-1]
        # [SHARD, w] -> [128, RT*w] with free = (tile, w)
        refc_t = np.ascontiguousarray(
            refc_c.reshape(RT, 128, 16).transpose(1, 0, 2).reshape(128, RT * 16)
        )
        refa_t = np.ascontiguousarray(
            refa_c.reshape(RT, 128, 112).transpose(1, 0, 2).reshape(128, RT * 112)
        )
        per_core.append(dict(
            shared,
            lhsT=np.ascontiguousarray(lhsT_full[:, sl]),
            bias=bias_t,
            selfidx=selfidx,
            refc=refc_t,
            refa=refa_t,
        ))
    return per_core


_NC_CACHE = {}


def run_cores(inputs, debug_out=False, **run_kwargs):
    key = debug_out
    if key not in _NC_CACHE:
        _NC_CACHE[key] = build_nc(debug_out=debug_out)
    nc = _NC_CACHE[key]
    in_maps = _prep_inputs(**inputs)
    res = run_bass_kernel_spmd(
        nc, in_maps, core_ids=list(range(NCORES)), **run_kwargs
    )
    return res


def kernel(embeddings, reference_curvature_sig, reference_angular_sig):
    res = run_cores(dict(
        embeddings=embeddings,
        reference_curvature_sig=reference_curvature_sig,
        reference_angular_sig=reference_angular_sig,
    ))
    css = 0.0
    ass = 0.0
    for r in res.results:
        css += float(r["partial"][0, 0])
        ass += float(r["partial"][0, 1])
    curv_loss = css / (N * 15)
    ang_loss = ass / (N * 105)
    out = np.float32(0.3 * curv_loss + 0.7 * ang_loss)
    return np.asarray(out, dtype=np.float32)


# revision 13
# speedup vs baseline: 1.0242x; 1.0242x over previous
"""Trainium2 Bass kernel for nn_CurvatureOnlyRegularizer (retrieval_knn).

Full inputs -> full output. Shards the 8192 points row-wise across 8 cores.

Per-core pipeline (1024 rows = 8 row-tiles of 128), software-pipelined per
tile:
  A. S = e1 . e2^T via bf16 PE matmul (4 K-chunks, 2 column-chunks of 512 per
     PSUM group).  ACT evacuates as t1 = Relu(psum*25 + bias_i) with bias_i
     folding the per-row term and the 1.5*2^23 magic constant, so t1 holds an
     integer m_i-part in fp32's integer binade.
  B. One scalar_tensor_tensor adds the per-chunk table (-25*n_j quantized +
     idx_j/512), producing packed = m + idx/512 with m = round(25*(C0-d^2)),
     |m| < 2^15 for every real neighbor so the packed value is exact.  Per-512-chunk max8 carries indices through selection for free;
     top-16-of-candidates + max_index recovers (chunk, idx) -> global idx.
  C. Curvature signature from the quantized d'^2; inv-distances stashed for
     the cosine stage.
  D. Neighbor embeddings gathered via dma_gather(transpose=True) into
     [D-partition, point*16] layout; PE gram (4 K-chunks x 16 col-groups)
     gives all pairwise dots incl. the self row/col.
  E. The gram PSUM is ACT-evacuated to fp16 and ONE 4-dim-AP DMA per (tile,
     half) scatters the 16x16 block-diagonals directly into per-point rows
     (ptR[point, l*16+m]) - no separate fold pass.
  F. cos = (G - G_l15 - G_r15 + G_1515) * invd_l * invd_m; upper-triangle via
     ap_gather; signatures sorted descending by max8/match_replace rounds and
     MSE'd against host-reversed references on ACT.  Phase F of tile t runs
     under phase A of tile t+2.
Host sums the 8 per-core partial sums.
"""

import os
from contextlib import ExitStack

import ml_dtypes
import numpy as np

import concourse.bass as bass
import concourse.bass_isa as bass_isa
import concourse.mybir as mybir
import concourse.tile as tile
from concourse import bacc
from concourse.bass import ds, ts
from concourse.bass_utils import run_bass_kernel_spmd

N, D, K = 8192, 512, 15
NCORES = 8
SHARD = N // NCORES            # 1024
RT = SHARD // 128              # 8 row-tiles per core
NCH = N // 512                 # 16 column chunks
NG = NCH // 2                  # 8 psum groups of 2 chunks per row-tile
MAGIC = 12582912.0             # 1.5 * 2^23
C0 = 2052.0
QSCALE = 25.0
PAD_CURV = -1.0
PAD_ANG = -4.0
NEG_BIG = -3.0e38
F32 = mybir.dt.float32
F16 = mybir.dt.float16
BF16 = mybir.dt.bfloat16
I16 = mybir.dt.int16
U32 = mybir.dt.uint32
AX = mybir.AxisListType
OP = mybir.AluOpType
AF = mybir.ActivationFunctionType

# which of the 8 psum groups run their pack-STT on gpsimd (rest on vector);
# gpsimd lacks the fused scalar_tensor_tensor opcode, so this must stay empty
STT_ON_GPSIMD = ()


def build_nc(debug_out: bool = False):
    nc = bacc.Bacc("TRN2", target_bir_lowering=False, debug=False)

    rhsT_d = nc.dram_tensor("rhsT", [D, N], BF16, kind="ExternalInput")
    lhsT_d = nc.dram_tensor("lhsT", [D, SHARD], BF16, kind="ExternalInput")
    egat_d = nc.dram_tensor("egather", [N, D], BF16, kind="ExternalInput")
    nji_d = nc.dram_tensor("njiota", [128, N], F32, kind="ExternalInput")
    bias_d = nc.dram_tensor("bias", [128, RT], F32, kind="ExternalInput")
    self_d = nc.dram_tensor("selfidx", [128, RT], F32, kind="ExternalInput")
    refc_d = nc.dram_tensor("refc", [128, RT * 16], F32, kind="ExternalInput")
    refa_d = nc.dram_tensor("refa", [128, RT * 112], F32, kind="ExternalInput")
    triu_d = nc.dram_tensor("triu", [128, 7], I16, kind="ExternalInput")
    fold_d = nc.dram_tensor("foldbuf", [RT, 128, 256], F16, kind="Internal")
    part_d = nc.dram_tensor("partial", [1, 2], F32, kind="ExternalOutput")
    if debug_out:
        dbg_idx_d = nc.dram_tensor("dbg_idx", [128, 16], F32, kind="ExternalOutput")
        dbg_d2_d = nc.dram_tensor("dbg_d2", [128, 16], F32, kind="ExternalOutput")
        dbg_srtc_d = nc.dram_tensor("dbg_srtc", [128, 16], F32, kind="ExternalOutput")
        dbg_ang_d = nc.dram_tensor("dbg_ang", [128, 112], F32, kind="ExternalOutput")
        dbg_cand_d = nc.dram_tensor("dbg_cand", [128, 128], F32, kind="ExternalOutput")
        dbg_ptr_d = nc.dram_tensor("dbg_ptr", [128, 256], F32, kind="ExternalOutput")

    # preamble (before Tile body): the gather-count register
    r2048 = nc.gpsimd.to_reg(2048)
    rfill1 = nc.gpsimd.to_reg(1.0)

    with tile.TileContext(nc) as tc, ExitStack() as ctx:
        const = ctx.enter_context(tc.tile_pool(name="const", bufs=1))
        sel = ctx.enter_context(tc.tile_pool(name="sel", bufs=3))
        scr = ctx.enter_context(tc.tile_pool(name="scr", bufs=2))
        fb = ctx.enter_context(tc.tile_pool(name="fb", bufs=2))
        gskp = ctx.enter_context(tc.tile_pool(name="gskp", bufs=2))
        vbuf = ctx.enter_context(tc.tile_pool(name="vbuf", bufs=3))
        ptrp = ctx.enter_context(tc.tile_pool(name="ptrp", bufs=3))
        ivdp = ctx.enter_context(tc.tile_pool(name="ivdp", bufs=4))
        psA = ctx.enter_context(tc.tile_pool(name="psA", bufs=2, space="PSUM"))
        psG = ctx.enter_context(tc.tile_pool(name="psG", bufs=2, space="PSUM"))
        psT = ctx.enter_context(tc.tile_pool(name="psT", bufs=1, space="PSUM"))

        # ---- constants / resident data ----
        rhs_sb = [const.tile([128, N], BF16, tag=f"rhs{c}", name=f"rhs{c}") for c in range(4)]
        lhs_sb = [const.tile([128, SHARD], BF16, tag=f"lhs{c}", name=f"lhs{c}") for c in range(4)]
        nji_sb = const.tile([128, N], F32, tag="nji")
        bias_sb = const.tile([128, RT], F32, tag="bias")
        self_sb = const.tile([128, RT], F32, tag="self")
        refc_sb = const.tile([128, RT * 16], F32, tag="refc")
        refa_sb = const.tile([128, RT * 112], F32, tag="refa")
        triu_sb = const.tile([128, 7], I16, tag="triu")
        perm = const.tile([128, 128], F32, tag="perm")
        repmat = const.tile([16, 128], F32, tag="repmat")
        css = const.tile([128, 1], F32, tag="css")
        ass = const.tile([128, 1], F32, tag="ass")

        for c in range(4):
            nc.sync.dma_start(rhs_sb[c][:], rhsT_d.ap()[ts(c, 128), :])
            nc.sync.dma_start(lhs_sb[c][:], lhsT_d.ap()[ts(c, 128), :])
        nc.sync.dma_start(nji_sb[:], nji_d.ap()[:])
        nc.sync.dma_start(bias_sb[:], bias_d.ap()[:])
        nc.sync.dma_start(self_sb[:], self_d.ap()[:])
        nc.sync.dma_start(triu_sb[:], triu_d.ap()[:])
        nc.sync.dma_start(refc_sb[:], refc_d.ap()[:])
        nc.sync.dma_start(refa_sb[:], refa_d.ap()[:])
        # perm[k, 8u+v] = 1 iff k == 16v+u  (gram-position permutation)
        nc.gpsimd.memset(perm[:], 0.0)
        nc.gpsimd.affine_select(
            out=bass.AP(tensor=perm[:].tensor, offset=0,
                        ap=[perm[:].ap[0], [8, 16], [1, 8]]),
            in_=bass.AP(tensor=perm[:].tensor, offset=0,
                        ap=[perm[:].ap[0], [8, 16], [1, 8]]),
            compare_op=OP.not_equal,
            fill=rfill1, base=0,
            pattern=[[-1, 16], [-16, 8]],
            channel_multiplier=1,
        )
        # repmat[k, 16a+s] = 1 iff k == s  (partition-block replicator)
        nc.gpsimd.memset(repmat[:], 0.0)
        nc.gpsimd.affine_select(
            out=bass.AP(tensor=repmat[:].tensor, offset=0,
                        ap=[repmat[:].ap[0], [16, 8], [1, 16]]),
            in_=bass.AP(tensor=repmat[:].tensor, offset=0,
                        ap=[repmat[:].ap[0], [16, 8], [1, 16]]),
            compare_op=OP.not_equal,
            fill=rfill1, base=0,
            pattern=[[0, 8], [-1, 16]],
            channel_multiplier=1,
        )
        nc.vector.memset(css[:], 0.0)
        nc.vector.memset(ass[:], 0.0)

        # =========== phase A: matmul + pack + chunk-max8 ===========
        def phase_a(t):
            cand = sel.tile([128, 128], F32, tag="cand")
            for g in range(NG):
                ps = psA.tile([128, 1024], F32, tag="psA", name="psA")
                for c in range(4):
                    for q in range(2):
                        nc.tensor.matmul(
                            ps[:, ts(q, 512)],
                            lhs_sb[c][:, ts(t, 128)],
                            rhs_sb[c][:, ts(2 * g + q, 512)],
                            start=(c == 0),
                            stop=(c == 3),
                        )
                t1 = scr.tile([128, 1024], F32, tag="t1")
                nc.scalar.activation(
                    t1[:], ps[:], AF.Relu,
                    bias=bias_sb[:, t : t + 1], scale=QSCALE,
                )
                t2 = scr.tile([128, 1024], F32, tag="t2")
                eng = nc.gpsimd if g in STT_ON_GPSIMD else nc.vector
                eng.scalar_tensor_tensor(
                    t2[:], t1[:], -MAGIC, nji_sb[:, ts(g, 1024)],
                    op0=OP.add, op1=OP.add,
                )
                for q in range(2):
                    nc.vector.max(cand[:, ts(2 * g + q, 8)], t2[:, ts(q, 512)])
            return cand

        # =========== phase B: select top-16 + unpack; C: curvature ==
        def phase_bc(t, cand):
            v16 = sel.tile([128, 16], F32, tag="v16")
            nc.vector.max(v16[:, 0:8], cand[:])
            candz = sel.tile([128, 128], F32, tag="candz")
            nc.vector.match_replace(candz[:], v16[:, 0:8], cand[:], NEG_BIG)
            nc.vector.max(v16[:, 8:16], candz[:])
            pos = sel.tile([128, 16], U32, tag="pos")
            nc.vector.max_index(pos[:, 0:8], v16[:, 0:8], cand[:])
            nc.vector.max_index(pos[:, 8:16], v16[:, 8:16], candz[:])
            chunk_u = sel.tile([128, 16], U32, tag="chunku")
            nc.vector.tensor_scalar(
                chunk_u[:], pos[:], 3, None, op0=OP.logical_shift_right
            )
            chunk_f = sel.tile([128, 16], F32, tag="chunkf")
            nc.vector.tensor_copy(chunk_f[:], chunk_u[:])
            # unpack m (integer part) via magic round
            s1 = sel.tile([128, 16], F32, tag="s1")
            nc.vector.tensor_scalar(
                s1[:], v16[:], -0.4990234375, None, op0=OP.add
            )
            wv = sel.tile([128, 16], F32, tag="wv")
            nc.scalar.activation(wv[:], s1[:], AF.Copy, bias=MAGIC, scale=1.0)
            m16 = sel.tile([128, 16], F32, tag="m16")
            nc.vector.tensor_scalar(m16[:], wv[:], -MAGIC, None, op0=OP.add)
            # frac = v16 - m16 = idx/512 ; gidx = chunk*512 + frac*512
            frac = sel.tile([128, 16], F32, tag="frac")
            nc.vector.scalar_tensor_tensor(
                frac[:], m16[:], -1.0, v16[:], op0=OP.mult, op1=OP.add
            )
            gidx = sel.tile([128, 16], F32, tag="gidx")
            nc.vector.tensor_tensor(gidx[:], chunk_f[:], frac[:], op=OP.add)
            nc.vector.tensor_scalar(gidx[:], gidx[:], 512.0, None, op0=OP.mult)
            # dp2 = C0 - m/QSCALE (slot 0 = self, dropped)
            dp2 = sel.tile([128, 16], F32, tag="dp2")
            nc.vector.tensor_scalar(
                dp2[:], m16[:], -1.0 / QSCALE, C0, op0=OP.mult, op1=OP.add
            )
            # ---- phase C: curvature ----
            d2re = sel.tile([128, 16], F32, tag="d2re")
            nc.vector.tensor_scalar_max(d2re[:, 0:15], dp2[:, 1:16], 1e-12)
            nc.vector.memset(d2re[:, 15:16], 1.0)
            dt_ = sel.tile([128, 16], F32, tag="dt")
            nc.scalar.sqrt(dt_[:], d2re[:])
            ivd = ivdp.tile([128, 16], F32, tag="ivd")
            nc.vector.reciprocal(ivd[:], dt_[:])
            dsum = sel.tile([128, 1], F32, tag="dsum")
            nc.vector.reduce_sum(dsum[:], dt_[:, 0:15], axis=AX.X)
            dmean = sel.tile([128, 1], F32, tag="dmean")
            nc.vector.tensor_scalar(
                dmean[:], dsum[:], 1.0 / 15.0, 1e-8, op0=OP.mult, op1=OP.add
            )
            ivm = sel.tile([128, 1], F32, tag="ivm")
            nc.vector.reciprocal(ivm[:], dmean[:])
            sig = sel.tile([128, 16], F32, tag="sig")
            nc.vector.tensor_scalar(
                sig[:, 0:15], dt_[:, 0:15], ivm[:], None, op0=OP.mult
            )
            nc.vector.memset(sig[:, 15:16], PAD_CURV)
            srtc = sel.tile([128, 16], F32, tag="srtc")
            nc.vector.max(srtc[:, 0:8], sig[:])
            sigz = sel.tile([128, 16], F32, tag="sigz")
            nc.vector.match_replace(sigz[:], srtc[:, 0:8], sig[:], -2.0)
            nc.vector.max(srtc[:, 8:16], sigz[:])
            dcv = sel.tile([128, 16], F32, tag="dcv")
            nc.vector.tensor_tensor(
                dcv[:], srtc[:], refc_sb[:, ts(t, 16)], op=OP.subtract
            )
            csq = sel.tile([128, 16], F32, tag="csq")
            css_t = sel.tile([128, 1], F32, tag="csst")
            nc.scalar.activation(csq[:], dcv[:], AF.Square, accum_out=css_t[:])
            nc.vector.tensor_tensor(css[:], css[:], css_t[:], op=OP.add)
            # ---- neighbor index tile for the gather ----
            kif = sel.tile([128, 16], F32, tag="kif")
            nc.vector.tensor_copy(kif[:, 0:15], gidx[:, 1:16])
            nc.vector.tensor_copy(kif[:, 15:16], self_sb[:, t : t + 1])
            # idx16[16a+s, Q] = kif[phi(Q), s]: transpose (perm rhs),
            # then replicate the 16-row block via a constant matmul.
            pst1 = psT.tile([16, 128], F32, tag="pst1", name="pst1")
            nc.tensor.transpose(pst1[:], kif[:], perm[:])
            t1s = sel.tile([16, 128], F32, tag="t1s")
            nc.vector.tensor_copy(t1s[:], pst1[:])
            pst = psT.tile([128, 128], F32, tag="pst", name="pst")
            nc.tensor.matmul(pst[:], repmat[:], t1s[:], start=True, stop=True)
            idx16 = sel.tile([128, 128], I16, tag="idx16")
            nc.vector.tensor_copy(idx16[:], pst[:])
            return ivd, kif, dp2, idx16, srtc, cand

        def gather(idx16):
            vt = vbuf.tile([128, 4, 2048], BF16, tag="vt")
            nc.gpsimd.dma_gather(
                out_ap=vt[:],
                in_ap=egat_d.ap()[:],
                idxs_ap=idx16[:],
                num_idxs=2048,
                num_idxs_reg=r2048,
                elem_size=512,
                transpose=True,
                single_packet=False,
            )
            return vt

        # =========== phase D/E: gram + fused fold scatter ===========
        def gram_fold(t, vt):
            ptr_t = ptrp.tile([128, 256], F16, tag="ptr")
            gsk = gskp.tile([128, 2048], F16, tag="gsk", name="gsk")
            for h in range(4):
                pg = psG.tile([128, 512], F32, tag="pg", name="pg")
                for g2h in range(4):
                    g2 = 4 * h + g2h
                    for c in range(4):
                        nc.tensor.matmul(
                            pg[:, ts(g2h, 128)],
                            vt[:, c, ts(g2, 128)],
                            vt[:, c, ts(g2, 128)],
                            start=(c == 0),
                            stop=(c == 3),
                        )
                nc.scalar.activation(gsk[:, ts(h, 512)], pg[:], AF.Copy)
            # fold via a DRAM bounce (DRAM APs have no partition-step rules):
            # per p: gsk[16p+l, 128*k + 16p + m] -> D[t, 16p+k, 16l+m]
            # (diag extraction + l<->k block transpose in one DMA), then one
            # contiguous DMA D[t] -> ptr_t.
            for p in range(8):
                src = bass.AP(
                    tensor=gsk[:].tensor,
                    offset=gsk[:].offset + p * (16 * 2048 + 16),
                    ap=[[2048, 16], [128, 16], [1, 16]],
                )
                dst = bass.AP(
                    tensor=fold_d.ap()[:].tensor,
                    offset=t * 128 * 256 + p * 16 * 256,
                    ap=[[16, 16], [256, 16], [1, 16]],
                )
                eng = nc.sync if p % 2 == 0 else nc.scalar
                eng.dma_start(dst, src)
            nc.sync.dma_start(ptr_t[:], fold_d.ap()[t])
            return ptr_t

        # =========== phase F: cos + sort + MSE ======================
        def phase_f(t, ptr_t, ivd, dbg=False):
            p0 = ptr_t[:].ap[0]
            base = ptr_t[:].offset
            pr = bass.AP(tensor=ptr_t[:].tensor, offset=base,
                         ap=[p0, [16, 16], [1, 16]])
            in_l15 = bass.AP(tensor=ptr_t[:].tensor, offset=base + 15,
                             ap=[p0, [16, 16], [0, 16]])
            in_r15 = bass.AP(tensor=ptr_t[:].tensor, offset=base + 240,
                             ap=[p0, [0, 16], [1, 16]])
            r1515 = bass.AP(tensor=ptr_t[:].tensor, offset=base + 255,
                            ap=[p0, [1, 1]])
            ta = fb.tile([128, 256], F32, tag="ta")
            nc.vector.tensor_tensor(ta[:], pr, in_l15, op=OP.subtract)
            tb = fb.tile([128, 256], F32, tag="tb")
            nc.vector.tensor_tensor(tb[:], ta[:], in_r15, op=OP.subtract)
            ivt = ivd[:]
            iv_l = bass.AP(
                tensor=ivt.tensor, offset=ivt.offset,
                ap=[ivt.ap[0], [1, 16], [0, 16]],
            )
            iv_m = bass.AP(
                tensor=ivt.tensor, offset=ivt.offset,
                ap=[ivt.ap[0], [0, 16], [1, 16]],
            )
            tcc = fb.tile([128, 256], F32, tag="tc")
            nc.vector.scalar_tensor_tensor(
                tcc[:], tb[:], r1515, iv_l, op0=OP.add, op1=OP.mult
            )
            cosv = fb.tile([128, 256], F32, tag="cosv")
            nc.vector.tensor_tensor(cosv[:], tcc[:], iv_m, op=OP.mult)
            angv = fb.tile([128, 112], F32, tag="angv")
            nc.gpsimd.ap_gather(
                out_ap=angv[:].rearrange("p (a b) -> p a b", b=1),
                in_ap=cosv[:].rearrange("p (a b) -> p a b", b=1),
                idxs_ap=triu_sb[:],
                channels=128,
                num_elems=256,
                d=1,
                num_idxs=112,
            )
            nc.vector.memset(angv[:, 105:112], PAD_ANG)
            srta = fb.tile([128, 112], F32, tag="srta")
            work = angv
            for r in range(14):
                nc.vector.max(srta[:, ts(r, 8)], work[:])
                if r < 13:
                    nwork = fb.tile([128, 112], F32, tag=f"work{r % 2}")
                    nc.vector.match_replace(
                        nwork[:], srta[:, ts(r, 8)], work[:], NEG_BIG
                    )
                    work = nwork
            dav = fb.tile([128, 112], F32, tag="dav")
            nc.vector.tensor_tensor(
                dav[:], srta[:], refa_sb[:, ts(t, 112)], op=OP.subtract
            )
            asq = fb.tile([128, 112], F32, tag="asq")
            ass_t = fb.tile([128, 1], F32, tag="asst")
            nc.scalar.activation(asq[:], dav[:], AF.Square, accum_out=ass_t[:])
            nc.vector.tensor_tensor(ass[:], ass[:], ass_t[:], op=OP.add)
            if dbg:
                nc.sync.dma_start(dbg_ang_d.ap()[:], srta[:])
                dbgp = fb.tile([128, 256], F32, tag="dbgp")
                nc.vector.tensor_copy(dbgp[:], ptr_t[:])
                nc.sync.dma_start(dbg_ptr_d.ap()[:], dbgp[:])

        # ================= main software-pipelined loop =============
        inflight = []   # list of (t, ptr_t, ivd) pending phase F
        gpend = []      # list of (t, vt) pending gram
        for t in range(RT):
            with nc.named_scope(f"A{t}"):
                cand = phase_a(t)
            with nc.named_scope(f"B{t}"):
                ivd, kif, dp2, idx16, srtc, cand_ = phase_bc(t, cand)
            with nc.named_scope(f"gth{t}"):
                vt = gather(idx16)
            if debug_out and t == 0:
                nc.sync.dma_start(dbg_idx_d.ap()[:], kif[:])
                nc.sync.dma_start(dbg_d2_d.ap()[:], dp2[:])
                nc.sync.dma_start(dbg_srtc_d.ap()[:], srtc[:])
                nc.sync.dma_start(dbg_cand_d.ap()[:], cand_[:])
            gpend.append((t, vt, ivd))
            if len(gpend) >= 3:
                tg, vtg, ivdg = gpend.pop(0)
                with nc.named_scope(f"G{tg}"):
                    ptr_t = gram_fold(tg, vtg)
                inflight.append((tg, ptr_t, ivdg))
            while len(inflight) >= 2:
                tf, ptr_f, ivd_f = inflight.pop(0)
                with nc.named_scope(f"F{tf}"):
                    phase_f(tf, ptr_f, ivd_f, dbg=(debug_out and tf == 0))
        while gpend:
            tg, vtg, ivdg = gpend.pop(0)
            with nc.named_scope(f"G{tg}"):
                ptr_t = gram_fold(tg, vtg)
            inflight.append((tg, ptr_t, ivdg))
        while inflight:
            tf, ptr_f, ivd_f = inflight.pop(0)
            with nc.named_scope(f"F{tf}"):
                phase_f(tf, ptr_f, ivd_f, dbg=(debug_out and tf == 0))

        # ---- final reduce + output ----
        cred = const.tile([128, 1], F32, tag="cred")
        ared = const.tile([128, 1], F32, tag="ared")
        nc.gpsimd.partition_all_reduce(
            cred[:], css[:], channels=128, reduce_op=bass_isa.ReduceOp.add
        )
        nc.gpsimd.partition_all_reduce(
            ared[:], ass[:], channels=128, reduce_op=bass_isa.ReduceOp.add
        )
        fin = const.tile([1, 2], F32, tag="fin")
        nc.vector.tensor_copy(fin[0:1, 0:1], cred[0:1, :])
        nc.vector.tensor_copy(fin[0:1, 1:2], ared[0:1, :])
        nc.sync.dma_start(part_d.ap()[:], fin[:])

    nc.compile()
    return nc


# =====================  host side  =====================

def _prep_inputs(embeddings, reference_curvature_sig, reference_angular_sig):
    emb32 = np.asarray(embeddings, dtype=np.float32)
    refc = np.asarray(reference_curvature_sig, dtype=np.float32)
    refa = np.asarray(reference_angular_sig, dtype=np.float32)

    e1_bf = (np.float32(np.sqrt(2.0)) * emb32).astype(ml_dtypes.bfloat16)
    e1 = e1_bf.astype(np.float32)
    e2_bf = (2.0 * e1).astype(ml_dtypes.bfloat16)       # exact x2
    n1 = np.sum(e1.astype(np.float64) * e1.astype(np.float64), axis=1).astype(
        np.float32
    )

    rhsT = np.ascontiguousarray(e2_bf.T)                # [512, N]
    lhsT_full = np.ascontiguousarray(e1_bf.T)           # [512, N]

    # packed per-chunk table: -round(25*n_j) + idx_within_chunk/512
    njq = np.round(QSCALE * n1.astype(np.float64))
    idx512 = np.tile(np.arange(512, dtype=np.float64) / 512.0, NCH)
    nji_row = (-njq + idx512).astype(np.float32)        # [N]
    nji = np.ascontiguousarray(
        np.broadcast_to(nji_row[None, :], (128, N))
    )

    tri = [l * 16 + m for l in range(15) for m in range(l + 1, 15)]
    tri += [255] * 7
    tri = np.array(tri, dtype=np.int16)                 # 112 entries
    triu = np.zeros((128, 7), dtype=np.int16)
    for p in range(128):
        for s in range(7):
            triu[p, s] = tri[s * 16 + (p & 15)]

    shared = dict(
        rhsT=rhsT, egather=e1_bf, njiota=nji, triu=triu,
    )
    per_core = []
    for c in range(NCORES):
        lo = c * SHARD
        sl = slice(lo, lo + SHARD)
        bias = (MAGIC + (C0 - n1[sl].astype(np.float64)) * QSCALE).astype(np.float32)
        bias_t = bias.reshape(RT, 128).T.copy()         # [128, RT]
        selfidx = (np.arange(lo, lo + SHARD, dtype=np.float32)
                   .reshape(RT, 128).T.copy())
        refc_c = np.full((SHARD, 16), PAD_CURV, dtype=np.float32)
        refc_c[:, 0:15] = refc[sl, ::-1]
        refa_c = np.full((SHARD, 112), PAD_ANG, dtype=np.float32)
        refa_c[:, 0:105] = refa[sl, ::-1]
        # [SHARD, w] -> [128, RT*w] with free = (tile, w)
        refc_t = np.ascontiguousarray(
            refc_c.reshape(RT, 128, 16).transpose(1, 0, 2).reshape(128, RT * 16)
        )
        refa_t = np.ascontiguousarray(
            refa_c.reshape(RT, 128, 112).transpose(1, 0, 2).reshape(128, RT * 112)
        )
        per_core.append(dict(
            shared,
            lhsT=np.ascontiguousarray(lhsT_full[:, sl]),
            bias=bias_t,
            selfidx=selfidx,
            refc=refc_t,
            refa=refa_t,
        ))
    return per_core


_NC_CACHE = {}


def run_cores(inputs, debug_out=False, **run_kwargs):
    key = debug_out
    if key not in _NC_CACHE:
        _NC_CACHE[key] = build_nc(debug_out=debug_out)
    nc = _NC_CACHE[key]
    in_maps = _prep_inputs(**inputs)
    res = run_bass_kernel_spmd(
        nc, in_maps, core_ids=list(range(NCORES)), **run_kwargs
    )
    return res


def kernel(embeddings, reference_curvature_sig, reference_angular_sig):
    res = run_cores(dict(
        embeddings=embeddings,
        reference_curvature_sig=reference_curvature_sig,
        reference_angular_sig=reference_angular_sig,
    ))
    css = 0.0
    ass = 0.0
    for r in res.results:
        css += float(r["partial"][0, 0])
        ass += float(r["partial"][0, 1])
    curv_loss = css / (N * 15)
    ang_loss = ass / (N * 105)
    out = np.float32(0.3 * curv_loss + 0.7 * ang_loss)
    return np.asarray(out, dtype=np.float32)


# revision 14
# speedup vs baseline: 1.0401x; 1.0155x over previous
"""Trainium2 Bass kernel for nn_CurvatureOnlyRegularizer (retrieval_knn).

Full inputs -> full output. Shards the 8192 points row-wise across 8 cores.

Per-core pipeline (1024 rows = 8 row-tiles of 128), software-pipelined per
tile:
  A. S = e1 . e2^T via bf16 PE matmul (4 K-chunks, 2 column-chunks of 512 per
     PSUM group).  ACT evacuates as t1 = Relu(psum*25 + bias_i) with bias_i
     folding the per-row term and the 1.5*2^23 magic constant, so t1 holds an
     integer m_i-part in fp32's integer binade.
  B. One scalar_tensor_tensor adds the per-chunk table (-25*n_j quantized +
     idx_j/512), producing packed = m + idx/512 with m = round(25*(C0-d^2)),
     |m| < 2^15 for every real neighbor so the packed value is exact.  Per-512-chunk max8 carries indices through selection for free;
     top-16-of-candidates + max_index recovers (chunk, idx) -> global idx.
  C. Curvature signature from the quantized d'^2; inv-distances stashed for
     the cosine stage.
  D. Neighbor embeddings gathered via dma_gather(transpose=True) into
     [D-partition, point*16] layout; PE gram (4 K-chunks x 16 col-groups)
     gives all pairwise dots incl. the self row/col.
  E. The gram PSUM is ACT-evacuated to fp16 and ONE 4-dim-AP DMA per (tile,
     half) scatters the 16x16 block-diagonals directly into per-point rows
     (ptR[point, l*16+m]) - no separate fold pass.
  F. cos = (G - G_l15 - G_r15 + G_1515) * invd_l * invd_m; upper-triangle via
     ap_gather; signatures sorted descending by max8/match_replace rounds and
     MSE'd against host-reversed references on ACT.  Phase F of tile t runs
     under phase A of tile t+2.
Host sums the 8 per-core partial sums.
"""

import os
from contextlib import ExitStack

import ml_dtypes
import numpy as np

import concourse.bass as bass
import concourse.bass_isa as bass_isa
import concourse.mybir as mybir
import concourse.tile as tile
from concourse import bacc
from concourse.bass import ds, ts
from concourse.bass_utils import run_bass_kernel_spmd

N, D, K = 8192, 512, 15
NCORES = 8
SHARD = N // NCORES            # 1024
RT = SHARD // 128              # 8 row-tiles per core
NCH = N // 512                 # 16 column chunks
NG = NCH // 2                  # 8 psum groups of 2 chunks per row-tile
MAGIC = 12582912.0             # 1.5 * 2^23
C0 = 2052.0
QSCALE = 25.0
PAD_CURV = -1.0
PAD_ANG = -4.0
NEG_BIG = -3.0e38
F32 = mybir.dt.float32
F16 = mybir.dt.float16
BF16 = mybir.dt.bfloat16
I16 = mybir.dt.int16
U32 = mybir.dt.uint32
AX = mybir.AxisListType
OP = mybir.AluOpType
AF = mybir.ActivationFunctionType

# which of the 8 psum groups run their pack-STT on gpsimd (rest on vector);
# gpsimd lacks the fused scalar_tensor_tensor opcode, so this must stay empty
STT_ON_GPSIMD = ()


def build_nc(debug_out: bool = False):
    nc = bacc.Bacc("TRN2", target_bir_lowering=False, debug=False)

    rhsT_d = nc.dram_tensor("rhsT", [D, N], BF16, kind="ExternalInput")
    lhsT_d = nc.dram_tensor("lhsT", [D, SHARD], BF16, kind="ExternalInput")
    egat_d = nc.dram_tensor("egather", [N, D], BF16, kind="ExternalInput")
    nji_d = nc.dram_tensor("njiota", [128, N], F32, kind="ExternalInput")
    bias_d = nc.dram_tensor("bias", [128, RT], F32, kind="ExternalInput")
    self_d = nc.dram_tensor("selfidx", [128, RT], F32, kind="ExternalInput")
    refc_d = nc.dram_tensor("refc", [128, RT * 16], F32, kind="ExternalInput")
    refa_d = nc.dram_tensor("refa", [128, RT * 112], F32, kind="ExternalInput")
    triu_d = nc.dram_tensor("triu", [128, 7], I16, kind="ExternalInput")
    fold_d = nc.dram_tensor("foldbuf", [RT, 128, 256], F16, kind="Internal")
    part_d = nc.dram_tensor("partial", [1, 2], F32, kind="ExternalOutput")
    if debug_out:
        dbg_idx_d = nc.dram_tensor("dbg_idx", [128, 16], F32, kind="ExternalOutput")
        dbg_d2_d = nc.dram_tensor("dbg_d2", [128, 16], F32, kind="ExternalOutput")
        dbg_srtc_d = nc.dram_tensor("dbg_srtc", [128, 16], F32, kind="ExternalOutput")
        dbg_ang_d = nc.dram_tensor("dbg_ang", [128, 112], F32, kind="ExternalOutput")
        dbg_cand_d = nc.dram_tensor("dbg_cand", [128, 128], F32, kind="ExternalOutput")
        dbg_ptr_d = nc.dram_tensor("dbg_ptr", [128, 256], F32, kind="ExternalOutput")

    # preamble (before Tile body): the gather-count register
    r2048 = nc.gpsimd.to_reg(2048)
    rfill1 = nc.gpsimd.to_reg(1.0)

    with tile.TileContext(nc) as tc, ExitStack() as ctx:
        const = ctx.enter_context(tc.tile_pool(name="const", bufs=1))
        sel = ctx.enter_context(tc.tile_pool(name="sel", bufs=3))
        scr = ctx.enter_context(tc.tile_pool(name="scr", bufs=2))
        fb = ctx.enter_context(tc.tile_pool(name="fb", bufs=2))
        gskp = ctx.enter_context(tc.tile_pool(name="gskp", bufs=2))
        vbuf = ctx.enter_context(tc.tile_pool(name="vbuf", bufs=3))
        ptrp = ctx.enter_context(tc.tile_pool(name="ptrp", bufs=3))
        ivdp = ctx.enter_context(tc.tile_pool(name="ivdp", bufs=4))
        psA = ctx.enter_context(tc.tile_pool(name="psA", bufs=2, space="PSUM"))
        psG = ctx.enter_context(tc.tile_pool(name="psG", bufs=2, space="PSUM"))
        psT = ctx.enter_context(tc.tile_pool(name="psT", bufs=1, space="PSUM"))

        # ---- constants / resident data ----
        rhs_sb = [const.tile([128, N], BF16, tag=f"rhs{c}", name=f"rhs{c}") for c in range(4)]
        lhs_sb = [const.tile([128, SHARD], BF16, tag=f"lhs{c}", name=f"lhs{c}") for c in range(4)]
        nji_sb = const.tile([128, N], F32, tag="nji")
        bias_sb = const.tile([128, RT], F32, tag="bias")
        self_sb = const.tile([128, RT], F32, tag="self")
        refc_sb = const.tile([128, RT * 16], F32, tag="refc")
        refa_sb = const.tile([128, RT * 112], F32, tag="refa")
        triu_sb = const.tile([128, 7], I16, tag="triu")
        perm = const.tile([128, 128], F32, tag="perm")
        repmat = const.tile([16, 128], F32, tag="repmat")
        css = const.tile([128, 1], F32, tag="css")
        ass = const.tile([128, 1], F32, tag="ass")

        for c in range(4):
            nc.sync.dma_start(rhs_sb[c][:], rhsT_d.ap()[ts(c, 128), :])
            nc.sync.dma_start(lhs_sb[c][:], lhsT_d.ap()[ts(c, 128), :])
        nc.sync.dma_start(nji_sb[:], nji_d.ap()[:])
        nc.sync.dma_start(bias_sb[:], bias_d.ap()[:])
        nc.sync.dma_start(self_sb[:], self_d.ap()[:])
        nc.sync.dma_start(triu_sb[:], triu_d.ap()[:])
        nc.sync.dma_start(refc_sb[:], refc_d.ap()[:])
        nc.sync.dma_start(refa_sb[:], refa_d.ap()[:])
        # perm[k, 8u+v] = 1 iff k == 16v+u  (gram-position permutation)
        nc.gpsimd.memset(perm[:], 0.0)
        nc.gpsimd.affine_select(
            out=bass.AP(tensor=perm[:].tensor, offset=0,
                        ap=[perm[:].ap[0], [8, 16], [1, 8]]),
            in_=bass.AP(tensor=perm[:].tensor, offset=0,
                        ap=[perm[:].ap[0], [8, 16], [1, 8]]),
            compare_op=OP.not_equal,
            fill=rfill1, base=0,
            pattern=[[-1, 16], [-16, 8]],
            channel_multiplier=1,
        )
        # repmat[k, 16a+s] = 1 iff k == s  (partition-block replicator)
        nc.gpsimd.memset(repmat[:], 0.0)
        nc.gpsimd.affine_select(
            out=bass.AP(tensor=repmat[:].tensor, offset=0,
                        ap=[repmat[:].ap[0], [16, 8], [1, 16]]),
            in_=bass.AP(tensor=repmat[:].tensor, offset=0,
                        ap=[repmat[:].ap[0], [16, 8], [1, 16]]),
            compare_op=OP.not_equal,
            fill=rfill1, base=0,
            pattern=[[0, 8], [-1, 16]],
            channel_multiplier=1,
        )
        nc.vector.memset(css[:], 0.0)
        nc.vector.memset(ass[:], 0.0)

        # =========== phase A: matmul + pack + chunk-max8 ===========
        def phase_a(t):
            cand = sel.tile([128, 128], F32, tag="cand")
            for g in range(NG):
                ps = psA.tile([128, 1024], F32, tag="psA", name="psA")
                for c in range(4):
                    for q in range(2):
                        nc.tensor.matmul(
                            ps[:, ts(q, 512)],
                            lhs_sb[c][:, ts(t, 128)],
                            rhs_sb[c][:, ts(2 * g + q, 512)],
                            start=(c == 0),
                            stop=(c == 3),
                        )
                t1 = scr.tile([128, 1024], F32, tag="t1")
                nc.scalar.activation(
                    t1[:], ps[:], AF.Relu,
                    bias=bias_sb[:, t : t + 1], scale=QSCALE,
                )
                t2 = scr.tile([128, 1024], F32, tag="t2")
                eng = nc.gpsimd if g in STT_ON_GPSIMD else nc.vector
                eng.scalar_tensor_tensor(
                    t2[:], t1[:], -MAGIC, nji_sb[:, ts(g, 1024)],
                    op0=OP.add, op1=OP.add,
                )
                for q in range(2):
                    nc.vector.max(cand[:, ts(2 * g + q, 8)], t2[:, ts(q, 512)])
            return cand

        # =========== phase B: select top-16 + unpack; C: curvature ==
        def phase_bc(t, cand):
            v16 = sel.tile([128, 16], F32, tag="v16")
            nc.vector.max(v16[:, 0:8], cand[:])
            candz = sel.tile([128, 128], F32, tag="candz")
            nc.vector.match_replace(candz[:], v16[:, 0:8], cand[:], NEG_BIG)
            nc.vector.max(v16[:, 8:16], candz[:])
            pos = sel.tile([128, 16], U32, tag="pos")
            nc.vector.max_index(pos[:, 0:8], v16[:, 0:8], cand[:])
            nc.vector.max_index(pos[:, 8:16], v16[:, 8:16], candz[:])
            chunk_u = sel.tile([128, 16], U32, tag="chunku")
            nc.vector.tensor_scalar(
                chunk_u[:], pos[:], 3, None, op0=OP.logical_shift_right
            )
            chunk_f = sel.tile([128, 16], F32, tag="chunkf")
            nc.vector.tensor_copy(chunk_f[:], chunk_u[:])
            # unpack m (integer part) via magic round
            s1 = sel.tile([128, 16], F32, tag="s1")
            nc.vector.tensor_scalar(
                s1[:], v16[:], -0.4990234375, None, op0=OP.add
            )
            wv = sel.tile([128, 16], F32, tag="wv")
            nc.scalar.activation(wv[:], s1[:], AF.Copy, bias=MAGIC, scale=1.0)
            m16 = sel.tile([128, 16], F32, tag="m16")
            nc.vector.tensor_scalar(m16[:], wv[:], -MAGIC, None, op0=OP.add)
            # frac = v16 - m16 = idx/512 ; gidx = chunk*512 + frac*512
            frac = sel.tile([128, 16], F32, tag="frac")
            nc.vector.scalar_tensor_tensor(
                frac[:], m16[:], -1.0, v16[:], op0=OP.mult, op1=OP.add
            )
            gidx = sel.tile([128, 16], F32, tag="gidx")
            nc.vector.tensor_tensor(gidx[:], chunk_f[:], frac[:], op=OP.add)
            nc.vector.tensor_scalar(gidx[:], gidx[:], 512.0, None, op0=OP.mult)
            # dp2 = C0 - m/QSCALE (slot 0 = self, dropped)
            dp2 = sel.tile([128, 16], F32, tag="dp2")
            nc.vector.tensor_scalar(
                dp2[:], m16[:], -1.0 / QSCALE, C0, op0=OP.mult, op1=OP.add
            )
            # ---- phase C: curvature ----
            d2re = sel.tile([128, 16], F32, tag="d2re")
            nc.vector.tensor_scalar_max(d2re[:, 0:15], dp2[:, 1:16], 1e-12)
            nc.vector.memset(d2re[:, 15:16], 1.0)
            dt_ = sel.tile([128, 16], F32, tag="dt")
            nc.scalar.sqrt(dt_[:], d2re[:])
            ivd = ivdp.tile([128, 16], F32, tag="ivd")
            nc.vector.reciprocal(ivd[:], dt_[:])
            dsum = sel.tile([128, 1], F32, tag="dsum")
            nc.vector.reduce_sum(dsum[:], dt_[:, 0:15], axis=AX.X)
            dmean = sel.tile([128, 1], F32, tag="dmean")
            nc.vector.tensor_scalar(
                dmean[:], dsum[:], 1.0 / 15.0, 1e-8, op0=OP.mult, op1=OP.add
            )
            ivm = sel.tile([128, 1], F32, tag="ivm")
            nc.vector.reciprocal(ivm[:], dmean[:])
            sig = sel.tile([128, 16], F32, tag="sig")
            nc.vector.tensor_scalar(
                sig[:, 0:15], dt_[:, 0:15], ivm[:], None, op0=OP.mult
            )
            nc.vector.memset(sig[:, 15:16], PAD_CURV)
            srtc = sel.tile([128, 16], F32, tag="srtc")
            nc.vector.max(srtc[:, 0:8], sig[:])
            sigz = sel.tile([128, 16], F32, tag="sigz")
            nc.vector.match_replace(sigz[:], srtc[:, 0:8], sig[:], -2.0)
            nc.vector.max(srtc[:, 8:16], sigz[:])
            dcv = sel.tile([128, 16], F32, tag="dcv")
            nc.vector.tensor_tensor(
                dcv[:], srtc[:], refc_sb[:, ts(t, 16)], op=OP.subtract
            )
            csq = sel.tile([128, 16], F32, tag="csq")
            css_t = sel.tile([128, 1], F32, tag="csst")
            nc.scalar.activation(csq[:], dcv[:], AF.Square, accum_out=css_t[:])
            nc.vector.tensor_tensor(css[:], css[:], css_t[:], op=OP.add)
            # ---- neighbor index tile for the gather ----
            kif = sel.tile([128, 16], F32, tag="kif")
            nc.vector.tensor_copy(kif[:, 0:15], gidx[:, 1:16])
            nc.vector.tensor_copy(kif[:, 15:16], self_sb[:, t : t + 1])
            # idx16[16a+s, Q] = kif[phi(Q), s]: transpose (perm rhs),
            # then replicate the 16-row block via a constant matmul.
            pst1 = psT.tile([16, 128], F32, tag="pst1", name="pst1")
            nc.tensor.transpose(pst1[:], kif[:], perm[:])
            t1s = sel.tile([16, 128], F32, tag="t1s")
            nc.vector.tensor_copy(t1s[:], pst1[:])
            pst = psT.tile([128, 128], F32, tag="pst", name="pst")
            nc.tensor.matmul(pst[:], repmat[:], t1s[:], start=True, stop=True)
            idx16 = sel.tile([128, 128], I16, tag="idx16")
            nc.vector.tensor_copy(idx16[:], pst[:])
            return ivd, kif, dp2, idx16, srtc, cand

        def gather(idx16):
            vt = vbuf.tile([128, 4, 2048], BF16, tag="vt")
            nc.gpsimd.dma_gather(
                out_ap=vt[:],
                in_ap=egat_d.ap()[:],
                idxs_ap=idx16[:],
                num_idxs=2048,
                num_idxs_reg=r2048,
                elem_size=512,
                transpose=True,
                single_packet=False,
            )
            return vt

        # =========== phase D/E: gram + fused fold scatter ===========
        def gram_fold(t, vt):
            ptr_t = ptrp.tile([128, 256], F16, tag="ptr")
            gsk = gskp.tile([128, 2048], F16, tag="gsk", name="gsk")
            for h in range(4):
                pg = psG.tile([128, 512], F32, tag="pg", name="pg")
                for g2h in range(4):
                    g2 = 4 * h + g2h
                    for c in range(4):
                        nc.tensor.matmul(
                            pg[:, ts(g2h, 128)],
                            vt[:, c, ts(g2, 128)],
                            vt[:, c, ts(g2, 128)],
                            start=(c == 0),
                            stop=(c == 3),
                        )
                nc.scalar.activation(gsk[:, ts(h, 512)], pg[:], AF.Copy)
            # fold via a DRAM bounce (DRAM APs have no partition-step rules):
            # per p: gsk[16p+l, 128*k + 16p + m] -> D[t, 16p+k, 16l+m]
            # (diag extraction + l<->k block transpose in one DMA), then one
            # contiguous DMA D[t] -> ptr_t.
            for p in range(8):
                src = bass.AP(
                    tensor=gsk[:].tensor,
                    offset=gsk[:].offset + p * (16 * 2048 + 16),
                    ap=[[2048, 16], [128, 16], [1, 16]],
                )
                dst = bass.AP(
                    tensor=fold_d.ap()[:].tensor,
                    offset=t * 128 * 256 + p * 16 * 256,
                    ap=[[16, 16], [256, 16], [1, 16]],
                )
                eng = nc.sync if p % 2 == 0 else nc.scalar
                eng.dma_start(dst, src)
            nc.sync.dma_start(ptr_t[:], fold_d.ap()[t])
            return ptr_t

        # =========== phase F: cos + sort + MSE ======================
        def phase_f(t, ptr_t, ivd, dbg=False):
            p0 = ptr_t[:].ap[0]
            base = ptr_t[:].offset
            pr = bass.AP(tensor=ptr_t[:].tensor, offset=base,
                         ap=[p0, [16, 16], [1, 16]])
            in_l15 = bass.AP(tensor=ptr_t[:].tensor, offset=base + 15,
                             ap=[p0, [16, 16], [0, 16]])
            in_r15 = bass.AP(tensor=ptr_t[:].tensor, offset=base + 240,
                             ap=[p0, [0, 16], [1, 16]])
            r1515 = bass.AP(tensor=ptr_t[:].tensor, offset=base + 255,
                            ap=[p0, [1, 1]])
            ta = fb.tile([128, 256], F32, tag="ta")
            nc.vector.tensor_tensor(ta[:], pr, in_l15, op=OP.subtract)
            tb = fb.tile([128, 256], F32, tag="tb")
            nc.vector.tensor_tensor(tb[:], ta[:], in_r15, op=OP.subtract)
            ivt = ivd[:]
            iv_l = bass.AP(
                tensor=ivt.tensor, offset=ivt.offset,
                ap=[ivt.ap[0], [1, 16], [0, 16]],
            )
            iv_m = bass.AP(
                tensor=ivt.tensor, offset=ivt.offset,
                ap=[ivt.ap[0], [0, 16], [1, 16]],
            )
            tcc = fb.tile([128, 256], F32, tag="tc")
            nc.vector.scalar_tensor_tensor(
                tcc[:], tb[:], r1515, iv_l, op0=OP.add, op1=OP.mult
            )
            cosv = fb.tile([128, 256], F32, tag="cosv")
            nc.vector.tensor_tensor(cosv[:], tcc[:], iv_m, op=OP.mult)
            angv = fb.tile([128, 112], F32, tag="angv")
            nc.gpsimd.ap_gather(
                out_ap=angv[:].rearrange("p (a b) -> p a b", b=1),
                in_ap=cosv[:].rearrange("p (a b) -> p a b", b=1),
                idxs_ap=triu_sb[:],
                channels=128,
                num_elems=256,
                d=1,
                num_idxs=112,
            )
            nc.vector.memset(angv[:, 105:112], PAD_ANG)
            srta = fb.tile([128, 112], F32, tag="srta")
            work = angv
            for r in range(14):
                nc.vector.max(srta[:, ts(r, 8)], work[:])
                if r < 13:
                    nwork = fb.tile([128, 112], F32, tag=f"work{r % 2}")
                    nc.vector.match_replace(
                        nwork[:], srta[:, ts(r, 8)], work[:], NEG_BIG
                    )
                    work = nwork
            dav = fb.tile([128, 112], F32, tag="dav")
            nc.vector.tensor_tensor(
                dav[:], srta[:], refa_sb[:, ts(t, 112)], op=OP.subtract
            )
            asq = fb.tile([128, 112], F32, tag="asq")
            ass_t = fb.tile([128, 1], F32, tag="asst")
            nc.scalar.activation(asq[:], dav[:], AF.Square, accum_out=ass_t[:])
            nc.vector.tensor_tensor(ass[:], ass[:], ass_t[:], op=OP.add)
            if dbg:
                nc.sync.dma_start(dbg_ang_d.ap()[:], srta[:])
                dbgp = fb.tile([128, 256], F32, tag="dbgp")
                nc.vector.tensor_copy(dbgp[:], ptr_t[:])
                nc.sync.dma_start(dbg_ptr_d.ap()[:], dbgp[:])

        # ================= main software-pipelined loop =============
        inflight = []   # list of (t, ptr_t, ivd) pending phase F
        gpend = []      # list of (t, vt, ivd) pending gram
        def drain_gram():
            tg, vtg, ivdg = gpend.pop(0)
            with nc.named_scope(f"G{tg}"):
                ptr_t = gram_fold(tg, vtg)
            inflight.append((tg, ptr_t, ivdg))
        def drain_f():
            tf, ptr_f, ivd_f = inflight.pop(0)
            with nc.named_scope(f"F{tf}"):
                phase_f(tf, ptr_f, ivd_f, dbg=(debug_out and tf == 0))
        for t in range(RT):
            # ready work first: gram of t-2, phase F of t-3 (all inputs
            # finished iterations ago, so queue heads never stall on them)
            if len(gpend) >= 3:
                drain_gram()
            while len(inflight) >= 2:
                drain_f()
            with nc.named_scope(f"A{t}"):
                cand = phase_a(t)
            with nc.named_scope(f"B{t}"):
                ivd, kif, dp2, idx16, srtc, cand_ = phase_bc(t, cand)
            with nc.named_scope(f"gth{t}"):
                vt = gather(idx16)
            if debug_out and t == 0:
                nc.sync.dma_start(dbg_idx_d.ap()[:], kif[:])
                nc.sync.dma_start(dbg_d2_d.ap()[:], dp2[:])
                nc.sync.dma_start(dbg_srtc_d.ap()[:], srtc[:])
                nc.sync.dma_start(dbg_cand_d.ap()[:], cand_[:])
            gpend.append((t, vt, ivd))
        while gpend:
            drain_gram()
            while len(inflight) >= 2:
                drain_f()
        while inflight:
            drain_f()

        # ---- final reduce + output ----
        cred = const.tile([128, 1], F32, tag="cred")
        ared = const.tile([128, 1], F32, tag="ared")
        nc.gpsimd.partition_all_reduce(
            cred[:], css[:], channels=128, reduce_op=bass_isa.ReduceOp.add
        )
        nc.gpsimd.partition_all_reduce(
            ared[:], ass[:], channels=128, reduce_op=bass_isa.ReduceOp.add
        )
        fin = const.tile([1, 2], F32, tag="fin")
        nc.vector.tensor_copy(fin[0:1, 0:1], cred[0:1, :])
        nc.vector.tensor_copy(fin[0:1, 1:2], ared[0:1, :])
        nc.sync.dma_start(part_d.ap()[:], fin[:])

    nc.compile()
    return nc


# =====================  host side  =====================

def _prep_inputs(embeddings, reference_curvature_sig, reference_angular_sig):
    emb32 = np.asarray(embeddings, dtype=np.float32)
    refc = np.asarray(reference_curvature_sig, dtype=np.float32)
    refa = np.asarray(reference_angular_sig, dtype=np.float32)

    e1_bf = (np.float32(np.sqrt(2.0)) * emb32).astype(ml_dtypes.bfloat16)
    e1 = e1_bf.astype(np.float32)
    e2_bf = (2.0 * e1).astype(ml_dtypes.bfloat16)       # exact x2
    n1 = np.sum(e1.astype(np.float64) * e1.astype(np.float64), axis=1).astype(
        np.float32
    )

    rhsT = np.ascontiguousarray(e2_bf.T)                # [512, N]
    lhsT_full = np.ascontiguousarray(e1_bf.T)           # [512, N]

    # packed per-chunk table: -round(25*n_j) + idx_within_chunk/512
    njq = np.round(QSCALE * n1.astype(np.float64))
    idx512 = np.tile(np.arange(512, dtype=np.float64) / 512.0, NCH)
    nji_row = (-njq + idx512).astype(np.float32)        # [N]
    nji = np.ascontiguousarray(
        np.broadcast_to(nji_row[None, :], (128, N))
    )

    tri = [l * 16 + m for l in range(15) for m in range(l + 1, 15)]
    tri += [255] * 7
    tri = np.array(tri, dtype=np.int16)                 # 112 entries
    triu = np.zeros((128, 7), dtype=np.int16)
    for p in range(128):
        for s in range(7):
            triu[p, s] = tri[s * 16 + (p & 15)]

    shared = dict(
        rhsT=rhsT, egather=e1_bf, njiota=nji, triu=triu,
    )
    per_core = []
    for c in range(NCORES):
        lo = c * SHARD
        sl = slice(lo, lo + SHARD)
        bias = (MAGIC + (C0 - n1[sl].astype(np.float64)) * QSCALE).astype(np.float32)
        bias_t = bias.reshape(RT, 128).T.copy()         # [128, RT]
        selfidx = (np.arange(lo, lo + SHARD, dtype=np.float32)
                   .reshape(RT, 128).T.copy())
        refc_c = np.full((SHARD, 16), PAD_CURV, dtype=np.float32)
        refc_c[:, 0:15] = refc[sl, ::-1]
        refa_c = np.full((SHARD, 112), PAD_ANG, dtype=np.float32)
        refa_c[:, 0:105] = refa[sl, ::-1]
        # [SHARD, w] -> [128, RT*w] with free = (tile, w)
        refc_t = np.ascontiguousarray(
            refc_c.reshape(RT, 128, 16).transpose(1, 0, 2).reshape(128, RT * 16)
        )
        refa_t = np.ascontiguousarray(
            refa_c.reshape(RT, 128, 112).transpose(1, 0, 2).reshape(128, RT * 112)
        )
        per_core.append(dict(
            shared,
            lhsT=np.ascontiguousarray(lhsT_full[:, sl]),
            bias=bias_t,
            selfidx=selfidx,
            refc=refc_t,
            refa=refa_t,
        ))
    return per_core


_NC_CACHE = {}


def run_cores(inputs, debug_out=False, **run_kwargs):
    key = debug_out
    if key not in _NC_CACHE:
        _NC_CACHE[key] = build_nc(debug_out=debug_out)
    nc = _NC_CACHE[key]
    in_maps = _prep_inputs(**inputs)
    res = run_bass_kernel_spmd(
        nc, in_maps, core_ids=list(range(NCORES)), **run_kwargs
    )
    return res


def kernel(embeddings, reference_curvature_sig, reference_angular_sig):
    res = run_cores(dict(
        embeddings=embeddings,
        reference_curvature_sig=reference_curvature_sig,
        reference_angular_sig=reference_angular_sig,
    ))
    css = 0.0
    ass = 0.0
    for r in res.results:
        css += float(r["partial"][0, 0])
        ass += float(r["partial"][0, 1])
    curv_loss = css / (N * 15)
    ang_loss = ass / (N * 105)
    out = np.float32(0.3 * curv_loss + 0.7 * ang_loss)
    return np.asarray(out, dtype=np.float32)


# revision 17
# speedup vs baseline: 1.0530x; 1.0124x over previous
"""Trainium2 Bass kernel for nn_CurvatureOnlyRegularizer (retrieval_knn).

Full inputs -> full output. Shards the 8192 points row-wise across 8 cores.

Per-core pipeline (1024 rows = 8 row-tiles of 128), software-pipelined per
tile:
  A. S = e1 . e2^T via bf16 PE matmul (4 K-chunks, 2 column-chunks of 512 per
     PSUM group).  ACT evacuates as t1 = Relu(psum*25 + bias_i) with bias_i
     folding the per-row term and the 1.5*2^23 magic constant, so t1 holds an
     integer m_i-part in fp32's integer binade.
  B. One scalar_tensor_tensor adds the per-chunk table (-25*n_j quantized +
     idx_j/512), producing packed = m + idx/512 with m = round(25*(C0-d^2)),
     |m| < 2^15 for every real neighbor so the packed value is exact.  Per-512-chunk max8 carries indices through selection for free;
     top-16-of-candidates + max_index recovers (chunk, idx) -> global idx.
  C. Curvature signature from the quantized d'^2; inv-distances stashed for
     the cosine stage.
  D. Neighbor embeddings gathered via dma_gather(transpose=True) into
     [D-partition, point*16] layout; PE gram (4 K-chunks x 16 col-groups)
     gives all pairwise dots incl. the self row/col.
  E. The gram PSUM is ACT-evacuated to fp16 and ONE 4-dim-AP DMA per (tile,
     half) scatters the 16x16 block-diagonals directly into per-point rows
     (ptR[point, l*16+m]) - no separate fold pass.
  F. cos = (G - G_l15 - G_r15 + G_1515) * invd_l * invd_m; upper-triangle via
     ap_gather; signatures sorted descending by max8/match_replace rounds and
     MSE'd against host-reversed references on ACT.  Phase F of tile t runs
     under phase A of tile t+2.
Host sums the 8 per-core partial sums.
"""

import os
from contextlib import ExitStack

import ml_dtypes
import numpy as np

import concourse.bass as bass
import concourse.bass_isa as bass_isa
import concourse.mybir as mybir
import concourse.tile as tile
from concourse import bacc
from concourse.bass import ds, ts
from concourse.bass_utils import run_bass_kernel_spmd

N, D, K = 8192, 512, 15
NCORES = 8
SHARD = N // NCORES            # 1024
RT = SHARD // 128              # 8 row-tiles per core
NCH = N // 512                 # 16 column chunks
NG = NCH // 2                  # 8 psum groups of 2 chunks per row-tile
MAGIC = 12582912.0             # 1.5 * 2^23
C0 = 2052.0
QSCALE = 25.0
PAD_CURV = -1.0
PAD_ANG = -4.0
NEG_BIG = -3.0e38
F32 = mybir.dt.float32
F16 = mybir.dt.float16
BF16 = mybir.dt.bfloat16
I16 = mybir.dt.int16
U32 = mybir.dt.uint32
AX = mybir.AxisListType
OP = mybir.AluOpType
AF = mybir.ActivationFunctionType

# which of the 8 psum groups run their pack-STT on gpsimd (rest on vector);
# gpsimd lacks the fused scalar_tensor_tensor opcode, so this must stay empty
STT_ON_GPSIMD = ()


def build_nc(debug_out: bool = False):
    nc = bacc.Bacc("TRN2", target_bir_lowering=False, debug=False)

    rhsT_d = nc.dram_tensor("rhsT", [D, N], BF16, kind="ExternalInput")
    lhsT_d = nc.dram_tensor("lhsT", [D, SHARD], BF16, kind="ExternalInput")
    egat_d = nc.dram_tensor("egather", [N, D], BF16, kind="ExternalInput")
    nji_d = nc.dram_tensor("njiota", [128, N], F32, kind="ExternalInput")
    bias_d = nc.dram_tensor("bias", [128, RT], F32, kind="ExternalInput")
    self_d = nc.dram_tensor("selfidx", [128, RT], F32, kind="ExternalInput")
    refc_d = nc.dram_tensor("refc", [128, RT * 16], F32, kind="ExternalInput")
    refa_d = nc.dram_tensor("refa", [128, RT * 112], F32, kind="ExternalInput")
    triu_d = nc.dram_tensor("triu", [128, 7], I16, kind="ExternalInput")
    fold_d = nc.dram_tensor("foldbuf", [RT, 128, 256], F16, kind="Internal")
    part_d = nc.dram_tensor("partial", [1, 2], F32, kind="ExternalOutput")
    if debug_out:
        dbg_idx_d = nc.dram_tensor("dbg_idx", [128, 16], F32, kind="ExternalOutput")
        dbg_d2_d = nc.dram_tensor("dbg_d2", [128, 16], F32, kind="ExternalOutput")
        dbg_srtc_d = nc.dram_tensor("dbg_srtc", [128, 16], F32, kind="ExternalOutput")
        dbg_ang_d = nc.dram_tensor("dbg_ang", [128, 112], F32, kind="ExternalOutput")
        dbg_cand_d = nc.dram_tensor("dbg_cand", [128, 128], F32, kind="ExternalOutput")
        dbg_ptr_d = nc.dram_tensor("dbg_ptr", [128, 256], F32, kind="ExternalOutput")

    # preamble (before Tile body): the gather-count register
    r2048 = nc.gpsimd.to_reg(2048)
    rfill1 = nc.gpsimd.to_reg(1.0)

    with tile.TileContext(nc) as tc, ExitStack() as ctx:
        const = ctx.enter_context(tc.tile_pool(name="const", bufs=1))
        sel = ctx.enter_context(tc.tile_pool(name="sel", bufs=3))
        scr = ctx.enter_context(tc.tile_pool(name="scr", bufs=2))
        fb = ctx.enter_context(tc.tile_pool(name="fb", bufs=2))
        gskp = ctx.enter_context(tc.tile_pool(name="gskp", bufs=2))
        vbuf = ctx.enter_context(tc.tile_pool(name="vbuf", bufs=3))
        ptrp = ctx.enter_context(tc.tile_pool(name="ptrp", bufs=3))
        ivdp = ctx.enter_context(tc.tile_pool(name="ivdp", bufs=5))
        psA = ctx.enter_context(tc.tile_pool(name="psA", bufs=2, space="PSUM"))
        psG = ctx.enter_context(tc.tile_pool(name="psG", bufs=2, space="PSUM"))
        psT = ctx.enter_context(tc.tile_pool(name="psT", bufs=1, space="PSUM"))

        # ---- constants / resident data ----
        rhs_sb = [const.tile([128, N], BF16, tag=f"rhs{c}", name=f"rhs{c}") for c in range(4)]
        lhs_sb = [const.tile([128, SHARD], BF16, tag=f"lhs{c}", name=f"lhs{c}") for c in range(4)]
        nji_sb = const.tile([128, N], F32, tag="nji")
        bias_sb = const.tile([128, RT], F32, tag="bias")
        self_sb = const.tile([128, RT], F32, tag="self")
        refc_sb = const.tile([128, RT * 16], F32, tag="refc")
        refa_sb = const.tile([128, RT * 112], F32, tag="refa")
        triu_sb = const.tile([128, 7], I16, tag="triu")
        perm = const.tile([128, 128], F32, tag="perm")
        repmat = const.tile([16, 128], F32, tag="repmat")
        css = const.tile([128, 1], F32, tag="css")
        ass = const.tile([128, 1], F32, tag="ass")

        for c in range(4):
            nc.sync.dma_start(rhs_sb[c][:], rhsT_d.ap()[ts(c, 128), :])
            nc.sync.dma_start(lhs_sb[c][:], lhsT_d.ap()[ts(c, 128), :])
        nc.sync.dma_start(nji_sb[:], nji_d.ap()[:])
        nc.sync.dma_start(bias_sb[:], bias_d.ap()[:])
        nc.sync.dma_start(self_sb[:], self_d.ap()[:])
        nc.sync.dma_start(triu_sb[:], triu_d.ap()[:])
        nc.sync.dma_start(refc_sb[:], refc_d.ap()[:])
        nc.sync.dma_start(refa_sb[:], refa_d.ap()[:])
        # perm[k, 8u+v] = 1 iff k == 16v+u  (gram-position permutation)
        nc.gpsimd.memset(perm[:], 0.0)
        nc.gpsimd.affine_select(
            out=bass.AP(tensor=perm[:].tensor, offset=0,
                        ap=[perm[:].ap[0], [8, 16], [1, 8]]),
            in_=bass.AP(tensor=perm[:].tensor, offset=0,
                        ap=[perm[:].ap[0], [8, 16], [1, 8]]),
            compare_op=OP.not_equal,
            fill=rfill1, base=0,
            pattern=[[-1, 16], [-16, 8]],
            channel_multiplier=1,
        )
        # repmat[k, 16a+s] = 1 iff k == s  (partition-block replicator)
        nc.gpsimd.memset(repmat[:], 0.0)
        nc.gpsimd.affine_select(
            out=bass.AP(tensor=repmat[:].tensor, offset=0,
                        ap=[repmat[:].ap[0], [16, 8], [1, 16]]),
            in_=bass.AP(tensor=repmat[:].tensor, offset=0,
                        ap=[repmat[:].ap[0], [16, 8], [1, 16]]),
            compare_op=OP.not_equal,
            fill=rfill1, base=0,
            pattern=[[0, 8], [-1, 16]],
            channel_multiplier=1,
        )
        nc.vector.memset(css[:], 0.0)
        nc.vector.memset(ass[:], 0.0)

        # =========== phase A: matmul + pack + chunk-max8 ===========
        def phase_a(t):
            cand = sel.tile([128, 128], F32, tag="cand")
            for g in range(NG):
                ps = psA.tile([128, 1024], F32, tag="psA", name="psA")
                for c in range(4):
                    for q in range(2):
                        nc.tensor.matmul(
                            ps[:, ts(q, 512)],
                            lhs_sb[c][:, ts(t, 128)],
                            rhs_sb[c][:, ts(2 * g + q, 512)],
                            start=(c == 0),
                            stop=(c == 3),
                        )
                t1 = scr.tile([128, 1024], F32, tag="t1")
                nc.scalar.activation(
                    t1[:], ps[:], AF.Relu,
                    bias=bias_sb[:, t : t + 1], scale=QSCALE,
                )
                t2 = scr.tile([128, 1024], F32, tag="t2")
                eng = nc.gpsimd if g in STT_ON_GPSIMD else nc.vector
                eng.scalar_tensor_tensor(
                    t2[:], t1[:], -MAGIC, nji_sb[:, ts(g, 1024)],
                    op0=OP.add, op1=OP.add,
                )
                for q in range(2):
                    nc.vector.max(cand[:, ts(2 * g + q, 8)], t2[:, ts(q, 512)])
            return cand

        # =========== phase B: select top-16 + unpack; C: curvature ==
        def phase_bc(t, cand):
            v16 = sel.tile([128, 16], F32, tag="v16")
            nc.vector.max(v16[:, 0:8], cand[:])
            candz = sel.tile([128, 128], F32, tag="candz")
            nc.vector.match_replace(candz[:], v16[:, 0:8], cand[:], NEG_BIG)
            nc.vector.max(v16[:, 8:16], candz[:])
            pos = sel.tile([128, 16], U32, tag="pos")
            nc.vector.max_index(pos[:, 0:8], v16[:, 0:8], cand[:])
            nc.vector.max_index(pos[:, 8:16], v16[:, 8:16], candz[:])
            chunk_u = sel.tile([128, 16], U32, tag="chunku")
            nc.vector.tensor_scalar(
                chunk_u[:], pos[:], 3, None, op0=OP.logical_shift_right
            )
            chunk512 = sel.tile([128, 16], F32, tag="chunk512")
            nc.vector.tensor_scalar(
                chunk512[:], chunk_u[:], 512.0, None, op0=OP.mult
            )
            # magic round on DVE: bias below the tie point, then the
            # +MAGIC/-MAGIC pair rounds to the integer m (slice outputs are
            # fp32-rounded, so the +MAGIC add quantizes)
            s1 = sel.tile([128, 16], F32, tag="s1")
            nc.vector.tensor_scalar(
                s1[:], v16[:], -0.4990234375, None, op0=OP.add
            )
            m16 = sel.tile([128, 16], F32, tag="m16")
            nc.vector.tensor_scalar(
                m16[:], s1[:], MAGIC, -MAGIC, op0=OP.add, op1=OP.add
            )
            # frac = v16 - m16 = idx/512; gidx = frac*512 + chunk*512
            frac = sel.tile([128, 16], F32, tag="frac")
            nc.vector.scalar_tensor_tensor(
                frac[:], m16[:], -1.0, v16[:], op0=OP.mult, op1=OP.add
            )
            gidx = sel.tile([128, 16], F32, tag="gidx")
            nc.vector.scalar_tensor_tensor(
                gidx[:], frac[:], 512.0, chunk512[:], op0=OP.mult, op1=OP.add
            )
            # dp2 = C0 - m/QSCALE (slot 0 = self, dropped)
            dp2 = sel.tile([128, 16], F32, tag="dp2")
            nc.vector.tensor_scalar(
                dp2[:], m16[:], -1.0 / QSCALE, C0, op0=OP.mult, op1=OP.add
            )
            # ---- phase C: curvature ----
            d2re = sel.tile([128, 16], F32, tag="d2re")
            nc.vector.tensor_scalar_max(d2re[:, 0:15], dp2[:, 1:16], 1e-12)
            nc.vector.memset(d2re[:, 15:16], 1.0)
            dt_ = sel.tile([128, 16], F32, tag="dt")
            nc.scalar.sqrt(dt_[:], d2re[:])
            ivd = ivdp.tile([128, 16], F32, tag="ivd")
            nc.vector.reciprocal(ivd[:], dt_[:])
            dsum = sel.tile([128, 1], F32, tag="dsum")
            nc.vector.reduce_sum(dsum[:], dt_[:, 0:15], axis=AX.X)
            dmean = sel.tile([128, 1], F32, tag="dmean")
            nc.vector.tensor_scalar(
                dmean[:], dsum[:], 1.0 / 15.0, 1e-8, op0=OP.mult, op1=OP.add
            )
            ivm = sel.tile([128, 1], F32, tag="ivm")
            nc.vector.reciprocal(ivm[:], dmean[:])
            sig = sel.tile([128, 16], F32, tag="sig")
            nc.vector.tensor_scalar(
                sig[:, 0:15], dt_[:, 0:15], ivm[:], None, op0=OP.mult
            )
            nc.vector.memset(sig[:, 15:16], PAD_CURV)
            srtc = sel.tile([128, 16], F32, tag="srtc")
            nc.vector.max(srtc[:, 0:8], sig[:])
            sigz = sel.tile([128, 16], F32, tag="sigz")
            nc.vector.match_replace(sigz[:], srtc[:, 0:8], sig[:], -2.0)
            nc.vector.max(srtc[:, 8:16], sigz[:])
            dcv = sel.tile([128, 16], F32, tag="dcv")
            nc.vector.tensor_tensor(
                dcv[:], srtc[:], refc_sb[:, ts(t, 16)], op=OP.subtract
            )
            csq = sel.tile([128, 16], F32, tag="csq")
            css_t = sel.tile([128, 1], F32, tag="csst")
            nc.scalar.activation(csq[:], dcv[:], AF.Square, accum_out=css_t[:])
            nc.vector.tensor_tensor(css[:], css[:], css_t[:], op=OP.add)
            # ---- neighbor index tile for the gather ----
            kif = sel.tile([128, 16], F32, tag="kif")
            nc.vector.tensor_copy(kif[:, 0:15], gidx[:, 1:16])
            nc.vector.tensor_copy(kif[:, 15:16], self_sb[:, t : t + 1])
            # idx16[16a+s, Q] = kif[phi(Q), s]: transpose (perm rhs),
            # then replicate the 16-row block via a constant matmul.
            pst1 = psT.tile([16, 128], F32, tag="pst1", name="pst1")
            nc.tensor.transpose(pst1[:], kif[:], perm[:])
            t1s = sel.tile([16, 128], F32, tag="t1s")
            nc.vector.tensor_copy(t1s[:], pst1[:])
            pst = psT.tile([128, 128], F32, tag="pst", name="pst")
            nc.tensor.matmul(pst[:], repmat[:], t1s[:], start=True, stop=True)
            idx16 = sel.tile([128, 128], I16, tag="idx16")
            nc.vector.tensor_copy(idx16[:], pst[:])
            return ivd, kif, dp2, idx16, srtc, cand

        def gather(idx16):
            vt = vbuf.tile([128, 4, 2048], BF16, tag="vt")
            nc.gpsimd.dma_gather(
                out_ap=vt[:],
                in_ap=egat_d.ap()[:],
                idxs_ap=idx16[:],
                num_idxs=2048,
                num_idxs_reg=r2048,
                elem_size=512,
                transpose=True,
                single_packet=False,
            )
            return vt

        # =========== phase D/E: gram + fused fold scatter ===========
        def gram_fold(t, vt):
            ptr_t = ptrp.tile([128, 256], F16, tag="ptr")
            gsk = gskp.tile([128, 2048], F16, tag="gsk", name="gsk")
            for h in range(4):
                pg = psG.tile([128, 512], F32, tag="pg", name="pg")
                for g2h in range(4):
                    g2 = 4 * h + g2h
                    for c in range(4):
                        nc.tensor.matmul(
                            pg[:, ts(g2h, 128)],
                            vt[:, c, ts(g2, 128)],
                            vt[:, c, ts(g2, 128)],
                            start=(c == 0),
                            stop=(c == 3),
                        )
                nc.scalar.activation(gsk[:, ts(h, 512)], pg[:], AF.Copy)
            # fold via a DRAM bounce (DRAM APs have no partition-step rules):
            # per p: gsk[16p+l, 128*k + 16p + m] -> D[t, 16p+k, 16l+m]
            # (diag extraction + l<->k block transpose in one DMA), then one
            # contiguous DMA D[t] -> ptr_t.
            for p in range(8):
                src = bass.AP(
                    tensor=gsk[:].tensor,
                    offset=gsk[:].offset + p * (16 * 2048 + 16),
                    ap=[[2048, 16], [128, 16], [1, 16]],
                )
                dst = bass.AP(
                    tensor=fold_d.ap()[:].tensor,
                    offset=t * 128 * 256 + p * 16 * 256,
                    ap=[[16, 16], [256, 16], [1, 16]],
                )
                eng = nc.sync if p % 2 == 0 else nc.scalar
                eng.dma_start(dst, src)
            nc.sync.dma_start(ptr_t[:], fold_d.ap()[t])
            return ptr_t

        # =========== phase F: cos + sort + MSE ======================
        def phase_f(t, ptr_t, ivd, dbg=False):
            p0 = ptr_t[:].ap[0]
            base = ptr_t[:].offset
            pr = bass.AP(tensor=ptr_t[:].tensor, offset=base,
                         ap=[p0, [16, 16], [1, 16]])
            in_l15 = bass.AP(tensor=ptr_t[:].tensor, offset=base + 15,
                             ap=[p0, [16, 16], [0, 16]])
            in_r15 = bass.AP(tensor=ptr_t[:].tensor, offset=base + 240,
                             ap=[p0, [0, 16], [1, 16]])
            r1515 = bass.AP(tensor=ptr_t[:].tensor, offset=base + 255,
                            ap=[p0, [1, 1]])
            ta = fb.tile([128, 256], F32, tag="ta")
            nc.vector.tensor_tensor(ta[:], pr, in_l15, op=OP.subtract)
            tb = fb.tile([128, 256], F32, tag="tb")
            nc.vector.tensor_tensor(tb[:], ta[:], in_r15, op=OP.subtract)
            ivt = ivd[:]
            iv_l = bass.AP(
                tensor=ivt.tensor, offset=ivt.offset,
                ap=[ivt.ap[0], [1, 16], [0, 16]],
            )
            iv_m = bass.AP(
                tensor=ivt.tensor, offset=ivt.offset,
                ap=[ivt.ap[0], [0, 16], [1, 16]],
            )
            tcc = fb.tile([128, 256], F32, tag="tc")
            nc.vector.scalar_tensor_tensor(
                tcc[:], tb[:], r1515, iv_l, op0=OP.add, op1=OP.mult
            )
            cosv = fb.tile([128, 256], F32, tag="cosv")
            nc.vector.tensor_tensor(cosv[:], tcc[:], iv_m, op=OP.mult)
            angv = fb.tile([128, 112], F32, tag="angv")
            nc.gpsimd.ap_gather(
                out_ap=angv[:].rearrange("p (a b) -> p a b", b=1),
                in_ap=cosv[:].rearrange("p (a b) -> p a b", b=1),
                idxs_ap=triu_sb[:],
                channels=128,
                num_elems=256,
                d=1,
                num_idxs=112,
            )
            nc.vector.memset(angv[:, 105:112], PAD_ANG)
            srta = fb.tile([128, 112], F32, tag="srta")
            work = angv
            for r in range(14):
                nc.vector.max(srta[:, ts(r, 8)], work[:])
                if r < 13:
                    nwork = fb.tile([128, 112], F32, tag=f"work{r % 2}")
                    nc.vector.match_replace(
                        nwork[:], srta[:, ts(r, 8)], work[:], NEG_BIG
                    )
                    work = nwork
            dav = fb.tile([128, 112], F32, tag="dav")
            nc.vector.tensor_tensor(
                dav[:], srta[:], refa_sb[:, ts(t, 112)], op=OP.subtract
            )
            asq = fb.tile([128, 112], F32, tag="asq")
            ass_t = fb.tile([128, 1], F32, tag="asst")
            nc.scalar.activation(asq[:], dav[:], AF.Square, accum_out=ass_t[:])
            nc.vector.tensor_tensor(ass[:], ass[:], ass_t[:], op=OP.add)
            if dbg:
                nc.sync.dma_start(dbg_ang_d.ap()[:], srta[:])
                dbgp = fb.tile([128, 256], F32, tag="dbgp")
                nc.vector.tensor_copy(dbgp[:], ptr_t[:])
                nc.sync.dma_start(dbg_ptr_d.ap()[:], dbgp[:])

        # ================= main software-pipelined loop =============
        gatherq = []    # (t, idx16, ivd) awaiting gather issue
        gpend = []      # (t, vt, ivd) awaiting gram
        inflight = []   # (t, ptr_t, ivd) awaiting phase F
        def drain_gather():
            tq, idxq, ivdq = gatherq.pop(0)
            with nc.named_scope(f"gth{tq}"):
                vt = gather(idxq)
            gpend.append((tq, vt, ivdq))
        def drain_gram():
            tg, vtg, ivdg = gpend.pop(0)
            with nc.named_scope(f"G{tg}"):
                ptr_t = gram_fold(tg, vtg)
            inflight.append((tg, ptr_t, ivdg))
        def drain_f():
            tf, ptr_f, ivd_f = inflight.pop(0)
            with nc.named_scope(f"F{tf}"):
                phase_f(tf, ptr_f, ivd_f, dbg=(debug_out and tf == 0))
        for t in range(RT):
            # Deferred-issue pipeline: every instruction emitted here has
            # inputs that completed in earlier iterations, so engine FIFO
            # heads never stall on long-latency producers.
            if gatherq:
                drain_gather()          # gather(t-1)
            if len(gpend) >= 3:
                drain_gram()            # gram(t-3)
            while len(inflight) >= 2:
                drain_f()               # phase F(t-4)
            with nc.named_scope(f"A{t}"):
                cand = phase_a(t)
            with nc.named_scope(f"B{t}"):
                ivd, kif, dp2, idx16, srtc, cand_ = phase_bc(t, cand)
            if debug_out and t == 0:
                nc.sync.dma_start(dbg_idx_d.ap()[:], kif[:])
                nc.sync.dma_start(dbg_d2_d.ap()[:], dp2[:])
                nc.sync.dma_start(dbg_srtc_d.ap()[:], srtc[:])
                nc.sync.dma_start(dbg_cand_d.ap()[:], cand_[:])
            gatherq.append((t, idx16, ivd))
        while gatherq:
            drain_gather()
        while gpend:
            drain_gram()
            while len(inflight) >= 2:
                drain_f()
        while inflight:
            drain_f()

        # ---- final reduce + output ----
        cred = const.tile([128, 1], F32, tag="cred")
        ared = const.tile([128, 1], F32, tag="ared")
        nc.gpsimd.partition_all_reduce(
            cred[:], css[:], channels=128, reduce_op=bass_isa.ReduceOp.add
        )
        nc.gpsimd.partition_all_reduce(
            ared[:], ass[:], channels=128, reduce_op=bass_isa.ReduceOp.add
        )
        fin = const.tile([1, 2], F32, tag="fin")
        nc.vector.tensor_copy(fin[0:1, 0:1], cred[0:1, :])
        nc.vector.tensor_copy(fin[0:1, 1:2], ared[0:1, :])
        nc.sync.dma_start(part_d.ap()[:], fin[:])

    nc.compile()
    return nc


# =====================  host side  =====================

def _prep_inputs(embeddings, reference_curvature_sig, reference_angular_sig):
    emb32 = np.asarray(embeddings, dtype=np.float32)
    refc = np.asarray(reference_curvature_sig, dtype=np.float32)
    refa = np.asarray(reference_angular_sig, dtype=np.float32)

    e1_bf = (np.float32(np.sqrt(2.0)) * emb32).astype(ml_dtypes.bfloat16)
    e1 = e1_bf.astype(np.float32)
    e2_bf = (2.0 * e1).astype(ml_dtypes.bfloat16)       # exact x2
    n1 = np.sum(e1.astype(np.float64) * e1.astype(np.float64), axis=1).astype(
        np.float32
    )

    rhsT = np.ascontiguousarray(e2_bf.T)                # [512, N]
    lhsT_full = np.ascontiguousarray(e1_bf.T)           # [512, N]

    # packed per-chunk table: -round(25*n_j) + idx_within_chunk/512
    njq = np.round(QSCALE * n1.astype(np.float64))
    idx512 = np.tile(np.arange(512, dtype=np.float64) / 512.0, NCH)
    nji_row = (-njq + idx512).astype(np.float32)        # [N]
    nji = np.ascontiguousarray(
        np.broadcast_to(nji_row[None, :], (128, N))
    )

    tri = [l * 16 + m for l in range(15) for m in range(l + 1, 15)]
    tri += [255] * 7
    tri = np.array(tri, dtype=np.int16)                 # 112 entries
    triu = np.zeros((128, 7), dtype=np.int16)
    for p in range(128):
        for s in range(7):
            triu[p, s] = tri[s * 16 + (p & 15)]

    shared = dict(
        rhsT=rhsT, egather=e1_bf, njiota=nji, triu=triu,
    )
    per_core = []
    for c in range(NCORES):
        lo = c * SHARD
        sl = slice(lo, lo + SHARD)
        bias = (MAGIC + (C0 - n1[sl].astype(np.float64)) * QSCALE).astype(np.float32)
        bias_t = bias.reshape(RT, 128).T.copy()         # [128, RT]
        selfidx = (np.arange(lo, lo + SHARD, dtype=np.float32)
                   .reshape(RT, 128).T.copy())
        refc_c = np.full((SHARD, 16), PAD_CURV, dtype=np.float32)
        refc_c[:, 0:15] = refc[sl, ::-1]
        refa_c = np.full((SHARD, 112), PAD_ANG, dtype=np.float32)
        refa_c[:, 0:105] = refa[sl, ::-1]
        # [SHARD, w] -> [128, RT*w] with free = (tile, w)
        refc_t = np.ascontiguousarray(
            refc_c.reshape(RT, 128, 16).transpose(1, 0, 2).reshape(128, RT * 16)
        )
        refa_t = np.ascontiguousarray(
            refa_c.reshape(RT, 128, 112).transpose(1, 0, 2).reshape(128, RT * 112)
        )
        per_core.append(dict(
            shared,
            lhsT=np.ascontiguousarray(lhsT_full[:, sl]),
            bias=bias_t,
            selfidx=selfidx,
            refc=refc_t,
            refa=refa_t,
        ))
    return per_core


_NC_CACHE = {}


def run_cores(inputs, debug_out=False, **run_kwargs):
    key = debug_out
    if key not in _NC_CACHE:
        _NC_CACHE[key] = build_nc(debug_out=debug_out)
    nc = _NC_CACHE[key]
    in_maps = _prep_inputs(**inputs)
    res = run_bass_kernel_spmd(
        nc, in_maps, core_ids=list(range(NCORES)), **run_kwargs
    )
    return res


def kernel(embeddings, reference_curvature_sig, reference_angular_sig):
    res = run_cores(dict(
        embeddings=embeddings,
        reference_curvature_sig=reference_curvature_sig,
        reference_angular_sig=reference_angular_sig,
    ))
    css = 0.0
    ass = 0.0
    for r in res.results:
        css += float(r["partial"][0, 0])
        ass += float(r["partial"][0, 1])
    curv_loss = css / (N * 15)
    ang_loss = ass / (N * 105)
    out = np.float32(0.3 * curv_loss + 0.7 * ang_loss)
    return np.asarray(out, dtype=np.float32)
